# revision 19
# baseline (speedup 1.0000x reference)
"""Trainium2 Bass kernel for CategoricalEntropyRegLoss.

Math: both loss terms factor so the [B,B] pairwise matrices are never built.

  feat_dists = sq_j + sq_k - 2 fn_j.fn_k            (rank FD+2)
  target_dists = (E_j - P_j.LQ_k) / D               (rank DC+1)
  S = sum_{jk} m_j m_k feat_dists * target_dists    (diag is exactly 0)
    = [ se*M + a*e - 2 Fe.F - Psq.L - Pbar.Lsq + 2 <U,V> ] / D
  tightness*M = a - sum_s ||seg_sum_s||^2 / max(cnt_s,1)

Everything needed is one matmul per core:
  out[1154, 258] = ext_seg^T @ ext_feat
  ext_seg  = [ onehot(code) | LQ | P | 1 | E ]      (B x 1154)
  ext_feat = [ m*fn | m | m*sq ]                    (B x 258)

Cross-core reduction: instead of an NRT AllReduce (~17-27us mesh-begin
latency + ~20-28us mesh algo for the 0.7-1.3MB payload), each core
broadcasts its partial directly into the 7 peers' SBUF via XOR-relative
remote_dma_broadcast (SPMD-safe: round r pairs core c with c^r, so every
receive buffer has exactly one writer), then reduces the 8 partials
locally and runs the (redundant) epilogue from SBUF. A prelude 1-byte
AllGather (bir_kernel_barrier) absorbs cross-core launch skew and
guarantees peers' semaphores are initialized before any send fires.

Precision: matmul operands fp16 (one-hot is exact; 2x PE throughput,
PSUM accumulates fp32). The 1024 segment rows travel fp16 (they only
feed the per-segment squared-norm term, errors average out over 1024
segments); the 130 stats rows travel fp32 because the diversity total
has ~7x cancellation and fp16 rounding of the large stats partials
costs ~1.6e-3 rel err (measured) vs ~1e-5 with fp32 stats.
"""

import numpy as np

B = 4096
FD = 256
C = 32
D = 2
NSEG = C ** D          # 1024
NCORES = 8
RB = B // NCORES       # 512 rows per core
KT = RB // 128         # 4 k-chunks of 128 rows
EF = FD + 2            # 258: [mfn | m | m*sq]
ES = NSEG + 2 * D * C + 2   # 1154: [onehot | LQ | P | ones | E]
NMT = (ES + 127) // 128     # 10 m-tiles (last has 2 rows)

_compiled = {}


def _build_bass():
    from contextlib import ExitStack
    import concourse.bass as bass
    import concourse.bacc as bacc
    import concourse.tile as tile
    from concourse import mybir

    from concourse.tile import add_dep_helper

    f32 = mybir.dt.float32
    f16 = mybir.dt.float16
    Alu = mybir.AluOpType
    Act = mybir.ActivationFunctionType
    Ax = mybir.AxisListType

    nc = bacc.Bacc(num_devices=NCORES)

    feat = nc.dram_tensor("features", [RB, FD], f32, kind="ExternalInput")
    targ = nc.dram_tensor("targets", [RB, D * C], f32, kind="ExternalInput")
    maskf = nc.dram_tensor("maskf", [RB, 1], f32, kind="ExternalInput")
    outd = nc.dram_tensor("out", [8], f32, kind="ExternalOutput")

    with ExitStack() as ctx:
        tc = ctx.enter_context(tile.TileContext(nc))
        consts = ctx.enter_context(tc.tile_pool(name="consts", bufs=1))
        work = ctx.enter_context(tc.tile_pool(name="work", bufs=1))
        keep = ctx.enter_context(tc.tile_pool(name="keep", bufs=1))
        psum = ctx.enter_context(tc.tile_pool(name="psum", bufs=1, space="PSUM"))

        # ---------------- constants ----------------
        ones128 = consts.tile([128, 1], f32)
        nc.vector.memset(ones128[:], 1.0)

        # ---- batched input loads spread over two queues ----
        tbig = keep.tile([128, KT, D * C], f32, name="tbig")
        nc.scalar.dma_start(
            out=tbig[:], in_=targ[:, :].rearrange("(a p) f -> p a f", p=128))
        mkbig = keep.tile([128, KT, 1], f32, name="mkbig")
        nc.scalar.dma_start(
            out=mkbig[:], in_=maskf[:, :].rearrange("(a p) f -> p a f", p=128))
        # two tiles (not halves of one) so chunk reads only wait their own DMA
        xbig0 = keep.tile([128, 2, FD], f32, name="xbig0")
        nc.sync.dma_start(
            out=xbig0[:],
            in_=feat[0:256, :].rearrange("(a p) f -> p a f", p=128))
        xbig1 = keep.tile([128, 2, FD], f32, name="xbig1")
        nc.gpsimd.dma_start(
            out=xbig1[:],
            in_=feat[256:512, :].rearrange("(a p) f -> p a f", p=128))

        def xchunk(kc):
            return xbig0[:, kc, :] if kc < 2 else xbig1[:, kc - 2, :]

        # iotas after the gpsimd input DMA trigger (not needed until ~15us)
        iota1024 = consts.tile([128, NSEG], f32)
        nc.gpsimd.iota(iota1024[:], [[1, NSEG]], channel_multiplier=0,
                       allow_small_or_imprecise_dtypes=True)
        # biota[j] = 32 - j  (for first-argmax via reduce_max)
        biota = consts.tile([128, C], f32)
        nc.gpsimd.iota(biota[:], [[-1, C]], base=C, channel_multiplier=0,
                       allow_small_or_imprecise_dtypes=True)

        NST = 2 * D * C + 2   # 130 stats columns: [lq | p | ones | E]
        es_oh = [keep.tile([128, NSEG], f16, name=f"esoh_{kc}")
                 for kc in range(KT)]
        es_st = [keep.tile([128, NST], f16, name=f"esst_{kc}")
                 for kc in range(KT)]
        ef_16 = [keep.tile([128, EF], f16, name=f"eff_{kc}")
                 for kc in range(KT)]

        # ---- ACT phase 1: row sum-of-squares (Square table loads once) ----
        sqpack = keep.tile([128, KT], f32, name="sqpack")
        scrsq = keep.tile([128, FD], f32, name="scrsq")
        act_chain = []
        for kc in range(KT):
            act_chain.append(nc.scalar.activation(
                out=scrsq[:], in_=xchunk(kc), func=Act.Square,
                accum_out=sqpack[:, kc:kc + 1]))
        # ---- ACT phase 2: one Sqrt for all chunks ----
        normpack = keep.tile([128, KT], f32, name="normpack")
        act_chain.append(nc.scalar.sqrt(normpack[:], sqpack[:]))
        nc.vector.tensor_scalar_max(out=normpack[:], in0=normpack[:],
                                    scalar1=1e-12)
        invpack = keep.tile([128, KT], f32, name="invpack")
        nc.vector.reciprocal(invpack[:], normpack[:])
        # minv = m * inv  (fold mask into the normalization scale)
        minvpack = keep.tile([128, KT], f32, name="minvpack")
        nc.vector.tensor_tensor(out=minvpack[:], in0=invpack[:],
                                in1=mkbig[:, :, 0], op=Alu.mult)

        # ---- targets chains (DVE) + Ln (ACT phase 3) ----
        # es_st columns: [0:64 lq | 64:128 p | 128 ones | 129 E]
        # chunk-batched front: one add / one reduce / one reciprocal
        t1big = keep.tile([128, KT, D * C], f32, name="t1big")
        nc.vector.tensor_scalar_add(out=t1big[:], in0=tbig[:], scalar1=1e-10)
        invsb = keep.tile([128, KT * D], f32, name="invsb")
        nc.vector.reduce_sum(
            out=invsb[:],
            in_=t1big[:].rearrange("p a (d c) -> p (a d) c", c=C),
            axis=Ax.X)
        nc.vector.reciprocal(invsb[:], invsb[:])
        ln_acts = []
        for kc in range(KT):
            st_t = es_st[kc]
            pt = st_t[:, D * C:2 * D * C]
            lqt = st_t[:, 0:D * C]
            for d_ in range(D):
                nc.vector.tensor_scalar_mul(
                    out=pt[:, C * d_:C * (d_ + 1)],
                    in0=t1big[:, kc, C * d_:C * (d_ + 1)],
                    scalar1=invsb[:, kc * D + d_:kc * D + d_ + 1])
            ln_acts.append(nc.scalar.activation(out=lqt, in_=pt,
                                                func=Act.Ln))

            # ---- first-argmax per dim, then code = cls0 + 32*cls1 ----
            cls = work.tile([128, D], f32, name=f"cls_{kc}", tag=f"cl_{kc}")
            for d_ in range(D):
                pch = pt[:, C * d_:C * (d_ + 1)]
                mx = work.tile([128, 1], f32, name=f"mx_{kc}_{d_}",
                               tag=f"mx_{kc}_{d_}")
                nc.vector.reduce_max(out=mx[:], in_=pch, axis=Ax.X)
                cand = work.tile([128, C], f32, name=f"cand_{kc}_{d_}",
                                 tag=f"cd_{kc}_{d_}")
                # (p == max) * (32 - idx); reduce_max -> 32 - first_argmax
                nc.vector.scalar_tensor_tensor(
                    out=cand[:], in0=pch, scalar=mx[:], in1=biota[:],
                    op0=Alu.is_equal, op1=Alu.mult)
                mq = work.tile([128, 1], f32, name=f"mq_{kc}_{d_}",
                               tag=f"mq_{kc}_{d_}")
                nc.vector.reduce_max(out=mq[:], in_=cand[:], axis=Ax.X)
                nc.vector.tensor_scalar(
                    out=cls[:, d_:d_ + 1], in0=mq[:], scalar1=-1.0,
                    scalar2=float(C), op0=Alu.mult, op1=Alu.add)
            code = work.tile([128, 1], f32, name=f"code_{kc}", tag=f"co_{kc}")
            nc.vector.tensor_scalar(
                out=code[:], in0=cls[:, 1:2], scalar1=float(C),
                scalar2=cls[:, 0:1], op0=Alu.mult, op1=Alu.add)
            # ---- one-hot (DVE; gpsimd runs this 20x slower AND port-starves
            # concurrent DVE ops — measured 15.6us per tile there) ----
            nc.vector.tensor_scalar(
                out=es_oh[kc][:], in0=iota1024[:], scalar1=code[:],
                scalar2=None, op0=Alu.is_equal)

        # ---- ext_feat = [x*(m*inv) | m | sq0*inv*minv] (ACT phase 4) ----
        copy_acts = []
        for kc in range(KT):
            ef_t = ef_16[kc]
            copy_acts.append(nc.scalar.activation(
                out=ef_t[:, 0:FD], in_=xchunk(kc), func=Act.Copy,
                scale=minvpack[:, kc:kc + 1]))
            nc.vector.tensor_copy(out=ef_t[:, FD:FD + 1], in_=mkbig[:, kc, :])
            nc.vector.tensor_scalar(out=ef_t[:, FD + 1:FD + 2],
                                    in0=sqpack[:, kc:kc + 1],
                                    scalar1=invpack[:, kc:kc + 1],
                                    scalar2=minvpack[:, kc:kc + 1],
                                    op0=Alu.mult, op1=Alu.mult)

        # E / ones columns, deferred: only the last two m-tiles need them
        for kc in range(KT):
            st_t = es_st[kc]
            scr64 = work.tile([128, D * C], f32, name=f"scr64_{kc}",
                              tag=f"s64_{kc}")
            nc.vector.tensor_tensor(out=scr64[:],
                                    in0=st_t[:, D * C:2 * D * C],
                                    in1=st_t[:, 0:D * C], op=Alu.mult)
            escr = work.tile([128, 1], f32, name=f"escr_{kc}",
                             tag=f"es_{kc}")
            nc.vector.reduce_sum(out=escr[:], in_=scr64[:], axis=Ax.X)
            nc.vector.tensor_copy(out=st_t[:, NST - 1:NST], in_=escr[:])
            nc.vector.memset(st_t[:, NST - 2:NST - 1], 1.0)

        # keep ACT ops grouped by function (avoid act-table reload thrash);
        # table-less Copies run before the Lns so ef is ready sooner
        act_chain = act_chain + copy_acts + ln_acts
        for a, b in zip(act_chain[1:], act_chain[:-1]):
            add_dep_helper(a.ins, b.ins, sync=False,
                           reason="act table grouping")

        # ---------------- exchange payload tiles ----------------
        # seg_pay: m-tiles 0..7 (segment rows), fp16.
        # st_pay: slot 0 = m-tile 8 ([LQ|P] rows), slot 1 = F row (p0) and
        #         E row (p1), fp32. Unused partitions zeroed.
        seg_pay = keep.tile([128, 8, EF], f16, name="seg_pay")
        st_pay = keep.tile([128, 2, EF], f32, name="st_pay")
        nc.vector.memset(st_pay[:, 1:2, :], 0.0)

        # ---------------- the one big matmul ----------------
        for mt in range(NMT):
            mlo = mt * 128
            msz = min(128, ES - mlo)
            ps = psum.tile([msz, EF], f32, name=f"ps_{mt}", tag=f"ps_{mt % 7}")
            for kc in range(KT):
                if mt < 8:
                    lhsT = es_oh[kc][:, mlo:mlo + msz]
                else:
                    lhsT = es_st[kc][:, mlo - NSEG:mlo - NSEG + msz]
                nc.tensor.matmul(out=ps[:], lhsT=lhsT, rhs=ef_16[kc][:],
                                 start=(kc == 0), stop=(kc == KT - 1))
            if mt < 8:
                # alternate engines so copies keep pace with the matmuls
                # (gpsimd cannot read PSUM; scalar's Copy is table-less)
                if mt % 2 == 0:
                    nc.vector.tensor_copy(out=seg_pay[:, mt, :], in_=ps[:])
                else:
                    nc.scalar.activation(out=seg_pay[:, mt, :], in_=ps[:],
                                         func=Act.Copy)
            elif mt == 8:
                nc.vector.tensor_copy(out=st_pay[:, 0, :], in_=ps[:])
            else:
                nc.vector.tensor_copy(out=st_pay[0:2, 1, :], in_=ps[0:2, :])

        # ---------------- direct SBUF exchange (replaces AllReduce) ------
        # Round r: every core sends its payload to peer (self XOR r). XOR is
        # a mutual pairing, so receive buffer r has exactly one writer and
        # the same program works on every core. Slot r of the 8-slot dest
        # list satisfies the D2D placement rule (slot bit2 == delta bit2).
        # Each call moves data on a distinct DMA-engine pair (lanes r, r+8),
        # so all 7 rounds stream concurrently.
        rsegs = [keep.tile([128, 8, EF], f16, name=f"rseg_{r}")
                 for r in range(1, NCORES)]
        rsts = [keep.tile([128, 2, EF], f32, name=f"rst_{r}")
                for r in range(1, NCORES)]
        lsem = nc.alloc_semaphore("xch_local")
        rsems = [nc.alloc_semaphore(f"xch_arrive_{r}")
                 for r in range(1, NCORES)]
        for r in range(1, NCORES):
            rdests = [None] * NCORES
            rdests[r] = (0, r)
            nc.gpsimd.remote_dma_broadcast(
                out_ap=rsegs[r - 1][:], in_ap=seg_pay[:],
                remote_sem=rsems[r - 1], local_sem=lsem, rdests=rdests)
            nc.gpsimd.remote_dma_broadcast(
                out_ap=rsts[r - 1][:], in_ap=st_pay[:],
                remote_sem=rsems[r - 1], local_sem=lsem, rdests=rdests)
        # peers must have cleared their semaphores (kernel entry) before any
        # send lands: barrier via the compile-time prelude 1-byte AllGather
        # (wait attached to the trigger post-scheduling; it also absorbs
        # cross-core launch skew concurrently with the compute phase above).
        nc._bir_kernel_barrier_sem_replica_groups.extend(
            [set(range(NCORES))])
        trig_op = nc.gpsimd.trigger_dma(count=None)

        # ---------------- local reduction of the 8 partials ----------------
        # seg adds on DVE (fp16), stats adds on gpsimd (fp32), concurrently.
        # Each first read of a receive buffer waits on that round's arrival
        # semaphore: +2 per call (2 DMA engines), 2 calls -> 4 per round.
        acc_seg = keep.tile([128, 8, EF], f16, name="acc_seg")
        acc_st = keep.tile([128, 2, EF], f32, name="acc_st")
        a01 = keep.tile([128, 8, EF], f16, name="a01")
        a23 = keep.tile([128, 8, EF], f16, name="a23")
        a45 = keep.tile([128, 8, EF], f16, name="a45")
        s01 = keep.tile([128, 2, EF], f32, name="s01")
        s23 = keep.tile([128, 2, EF], f32, name="s23")
        s45 = keep.tile([128, 2, EF], f32, name="s45")

        # arrival waits are attached AFTER TileContext scheduling (the
        # single-core scheduling sim cannot model peer-driven semaphore
        # increments and would report a deadlock); collect them here.
        post_waits = []

        def wge(op, r):
            post_waits.append((op, r))
            return op

        # tree: (p+r1) (r2+r3) (r4+r5) (r6+r7) -> pairwise -> acc
        wge(nc.vector.tensor_tensor(out=a01[:], in0=seg_pay[:],
                                    in1=rsegs[0][:], op=Alu.add), 1)
        wge(nc.vector.tensor_tensor(out=a23[:], in0=rsegs[1][:],
                                    in1=rsegs[2][:], op=Alu.add), 3)
        wge(nc.vector.tensor_tensor(out=a45[:], in0=rsegs[3][:],
                                    in1=rsegs[4][:], op=Alu.add), 5)
        t67 = nc.vector.tensor_tensor(out=a01[:], in0=a01[:],
                                      in1=a23[:], op=Alu.add)
        wge(nc.vector.tensor_tensor(out=a23[:], in0=rsegs[5][:],
                                    in1=rsegs[6][:], op=Alu.add), 7)
        nc.vector.tensor_tensor(out=a45[:], in0=a45[:], in1=a23[:],
                                op=Alu.add)
        nc.vector.tensor_tensor(out=acc_seg[:], in0=a01[:], in1=a45[:],
                                op=Alu.add)
        wge(nc.gpsimd.tensor_tensor(out=s01[:], in0=st_pay[:],
                                    in1=rsts[0][:], op=Alu.add), 1)
        wge(nc.gpsimd.tensor_tensor(out=s23[:], in0=rsts[1][:],
                                    in1=rsts[2][:], op=Alu.add), 3)
        wge(nc.gpsimd.tensor_tensor(out=s45[:], in0=rsts[3][:],
                                    in1=rsts[4][:], op=Alu.add), 5)
        nc.gpsimd.tensor_tensor(out=s01[:], in0=s01[:], in1=s23[:],
                                op=Alu.add)
        wge(nc.gpsimd.tensor_tensor(out=s23[:], in0=rsts[5][:],
                                    in1=rsts[6][:], op=Alu.add), 7)
        nc.gpsimd.tensor_tensor(out=s45[:], in0=s45[:], in1=s23[:],
                                op=Alu.add)
        nc.gpsimd.tensor_tensor(out=acc_st[:], in0=s01[:], in1=s45[:],
                                op=Alu.add)

        # ---------------- epilogue (redundant on every core) ----------------
        # stats helper rows re-based to partition 0 via tiny SBUF DMAs
        vt2 = keep.tile([64, EF], f32, name="vt2")
        nc.sync.dma_start(out=vt2[:], in_=acc_st[64:128, 0, :])
        e_row = keep.tile([1, EF], f32, name="e_row")
        nc.sync.dma_start(out=e_row[:], in_=acc_st[1:2, 1, :])

        Z = keep.tile([128, 8], f32, name="Z")
        nc.vector.memset(Z[:], 0.0)
        nrmp = keep.tile([128, 8], f32, name="nrmp")
        cdp = keep.tile([128, 8], f32, name="cdp")
        # segment squared-norms: slots 0..3 on ACT (Square+accum),
        # slots 4..7 on DVE (mult+reduce) — concurrent
        sq_acts = []
        for s in range(4):
            sq_acts.append(nc.scalar.activation(
                out=scrsq[:], in_=acc_seg[:, s, 0:FD], func=Act.Square,
                accum_out=nrmp[:, s:s + 1]))
        for a, b in zip(sq_acts[1:], sq_acts[:-1]):
            add_dep_helper(a.ins, b.ins, sync=False, reason="act grouping")
        scrB = keep.tile([128, 4, FD], f32, name="scrB")
        nc.vector.tensor_tensor(out=scrB[:], in0=acc_seg[:, 4:8, 0:FD],
                                in1=acc_seg[:, 4:8, 0:FD], op=Alu.mult)
        nc.vector.reduce_sum(out=nrmp[:, 4:8], in_=scrB[:], axis=Ax.X)
        nc.vector.tensor_scalar_max(out=cdp[:], in0=acc_seg[:, :, FD],
                                    scalar1=1.0)
        rcdp = keep.tile([128, 8], f32, name="rcdp")
        nc.vector.reciprocal(rcdp[:], cdp[:])
        termp = keep.tile([128, 8], f32, name="termp")
        nc.vector.tensor_tensor(out=termp[:], in0=nrmp[:], in1=rcdp[:],
                                op=Alu.mult)
        nc.vector.reduce_sum(out=Z[:, 0:1], in_=termp[:], axis=Ax.X)

        # stats: ut = LQ-block rows (partitions 0..63 of st slot 0)
        ut = acc_st[0:64, 0, :]
        scrU = keep.tile([64, FD], f32, name="scrU")
        nc.vector.tensor_tensor(out=scrU[:], in0=ut[:, 0:FD],
                                in1=vt2[:, 0:FD], op=Alu.mult)
        nc.vector.reduce_sum(out=Z[0:64, 1:2], in_=scrU[:], axis=Ax.X)
        nc.vector.tensor_tensor(out=Z[0:64, 2:3], in0=vt2[:, FD + 1:FD + 2],
                                in1=ut[:, FD:FD + 1], op=Alu.mult)     # Psq*L
        nc.vector.tensor_tensor(out=Z[0:64, 3:4], in0=vt2[:, FD:FD + 1],
                                in1=ut[:, FD + 1:FD + 2], op=Alu.mult)  # Pbar*Lsq
        scrF = keep.tile([1, FD], f32, name="scrF")
        nc.vector.tensor_tensor(out=scrF[:], in0=acc_st[0:1, 1, 0:FD],
                                in1=e_row[0:1, 0:FD], op=Alu.mult)
        nc.vector.reduce_sum(out=Z[0:1, 4:5], in_=scrF[:], axis=Ax.X)  # Fe.F

        zred = psum.tile([1, 8], f32, name="zred", tag="ps_0")
        nc.tensor.matmul(out=zred[:], lhsT=ones128[:], rhs=Z[:],
                         start=True, stop=True)
        zs = keep.tile([1, 8], f32, name="zs")
        nc.vector.tensor_copy(out=zs[:], in_=zred[:])

        # scalars: M=F[256], a=F[257], e=E[256], se=E[257] (all fp32)
        Mv = acc_st[0:1, 1, FD:FD + 1]
        av = acc_st[0:1, 1, FD + 1:FD + 2]
        ev = e_row[0:1, FD:FD + 1]
        sev = e_row[0:1, FD + 1:FD + 2]
        s_center = zs[0:1, 0:1]
        uv = zs[0:1, 1:2]
        psql = zs[0:1, 2:3]
        pbarlsq = zs[0:1, 3:4]
        fef = zs[0:1, 4:5]

        fin = keep.tile([1, 16], f32, name="fin")
        t_ = lambda i: fin[0:1, i:i + 1]
        # f0 = se*M ; f1 = a*e ; f2 = f0+f1
        nc.vector.tensor_tensor(out=t_(8), in0=sev, in1=Mv, op=Alu.mult)
        nc.vector.tensor_tensor(out=t_(9), in0=av, in1=ev, op=Alu.mult)
        nc.vector.tensor_tensor(out=t_(10), in0=t_(8), in1=t_(9), op=Alu.add)
        # f3 = -2*fef + f2
        nc.vector.tensor_scalar(out=t_(11), in0=fef, scalar1=-2.0,
                                scalar2=t_(10), op0=Alu.mult, op1=Alu.add)
        # f4 = f3 - psql ; f5 = f4 - pbarlsq
        nc.vector.tensor_tensor(out=t_(12), in0=t_(11), in1=psql, op=Alu.subtract)
        nc.vector.tensor_tensor(out=t_(13), in0=t_(12), in1=pbarlsq, op=Alu.subtract)
        # SD = 2*uv + f5
        nc.vector.tensor_scalar(out=t_(14), in0=uv, scalar1=2.0,
                                scalar2=t_(13), op0=Alu.mult, op1=Alu.add)
        # md = M*(M-1) ; rmd = 1/md ; div = SD*rmd*(-1/D)
        nc.vector.tensor_scalar(out=t_(15), in0=Mv, scalar1=-1.0,
                                scalar2=Mv, op0=Alu.add, op1=Alu.mult)
        nc.vector.reciprocal(t_(15), t_(15))
        nc.vector.tensor_tensor(out=t_(1), in0=t_(14), in1=t_(15), op=Alu.mult)
        nc.vector.tensor_scalar_mul(out=t_(1), in0=t_(1), scalar1=-1.0 / D)
        # tight = (a - s_center)/M
        nc.vector.tensor_tensor(out=t_(7), in0=av, in1=s_center, op=Alu.subtract)
        nc.vector.reciprocal(t_(6), Mv)
        nc.vector.tensor_tensor(out=t_(2), in0=t_(7), in1=t_(6), op=Alu.mult)
        # total = 0.1*div + 0.1*tight
        nc.vector.tensor_tensor(out=t_(0), in0=t_(1), in1=t_(2), op=Alu.add)
        nc.vector.tensor_scalar_mul(out=t_(0), in0=t_(0), scalar1=0.1)
        # debug slots
        nc.vector.tensor_copy(out=t_(3), in_=Mv)
        nc.vector.tensor_copy(out=t_(4), in_=av)
        nc.vector.tensor_copy(out=t_(5), in_=sev)

        nc.sync.dma_start(out=outd[None, :], in_=fin[0:1, 0:8])

    # TileContext has exited (scheduling sim done) — now attach the runtime
    # gates the sim could not model: each reduction's first read of a peer
    # buffer waits for that round's arrival semaphore (+2 per call x 2
    # calls), and the send trigger waits for the kernel-entry barrier.
    for op, r in post_waits:
        op.wait_op(rsems[r - 1], 4, "sem-ge", check=False)
    assert nc._bir_kernel_barrier_sem is not None
    trig_op.wait_op(nc._bir_kernel_barrier_sem,
                    nc.bir_kernel_barrier_sem_inc, "sem-ge", check=False)

    nc.finalize()
    return nc


def _get_compiled():
    if "nc" not in _compiled:
        _compiled["nc"] = _build_bass()
    return _compiled["nc"]


def _make_in_maps(features, targets, mask):
    features = np.ascontiguousarray(np.asarray(features, dtype=np.float32))
    targets = np.ascontiguousarray(np.asarray(targets, dtype=np.float32))
    maskf = np.asarray(mask).astype(np.float32).reshape(B, 1)
    in_maps = []
    for i in range(NCORES):
        sl = slice(i * RB, (i + 1) * RB)
        in_maps.append({
            "features": features[sl],
            "targets": targets[sl],
            "maskf": np.ascontiguousarray(maskf[sl]),
        })
    return in_maps


def kernel(features, targets, mask):
    from concourse.bass_utils import run_bass_kernel_spmd

    nc = _get_compiled()
    in_maps = _make_in_maps(features, targets, mask)
    res = run_bass_kernel_spmd(nc, in_maps, list(range(NCORES)))
    out = res.results[0]["out"]
    total = np.float32(out[0])
    diversity = np.float32(out[1])
    tightness = np.float32(out[2])
    return total, diversity, tightness


# revision 27
# speedup vs baseline: 1.1070x; 1.1070x over previous
"""Trainium2 Bass kernel for CategoricalEntropyRegLoss.

Math: both loss terms factor so the [B,B] pairwise matrices are never built.

  feat_dists = sq_j + sq_k - 2 fn_j.fn_k            (rank FD+2)
  target_dists = (E_j - P_j.LQ_k) / D               (rank DC+1)
  S = sum_{jk} m_j m_k feat_dists * target_dists    (diag is exactly 0)
    = [ se*M + a*e - 2 Fe.F - Psq.L - Pbar.Lsq + 2 <U,V> ] / D
  tightness*M = a - sum_s ||seg_sum_s||^2 / max(cnt_s,1)

Everything needed is one matmul per core:
  out[1154, 258] = ext_seg^T @ ext_feat
  ext_seg  = [ onehot(code) | LQ | P | 1 | E ]      (B x 1154)
  ext_feat = [ m*fn | m | m*sq ]                    (B x 258)

Cross-core reduction, in three overlapped pieces:

1. Stats rows (130 x 258, fp32): one small NRT AllReduce (264KB). Its
   completion doubles as the kernel-entry barrier: every core triggered
   the collective, hence passed its preamble (semaphore clears), so
   remote SBUF writes are safe afterwards.
2. Segment rows (1024 x 258): XOR reduce-scatter via remote_dma_broadcast.
   Each core's one-hot uses a host-supplied permuted iota (column j <->
   segment j XOR 128c), so TRUE segment block p lands in m-tile slot
   p XOR c. In round r every core sends slot r (66KB fp16) to peer c^r:
   slot = (dest block) XOR (own id) = r is compile-time constant, making
   a true reduce-scatter SPMD-expressible with static access patterns.
   Each core then owns the fully-reduced 128 segments of block c and
   computes its per-partition share of the tightness center term.
   (A v1 full-payload allgather exchange hit the ~45GB/s per-core DMA
   ceiling: 5.5MB took ~120us. This ships 0.46MB.)
3. The per-segment scalar partials ([128,1] fp32) take a second tiny
   XOR broadcast round; every core sums the 8 vectors elementwise and
   collapses partitions with the ones-matmul already used for Z.

Precision: matmul operands fp16 (one-hot exact, 2x PE rate, PSUM fp32).
Segment rows travel fp16 (feeds only the squared-norm term; error
averages over 1024 segments). Stats travel fp32: the diversity total
has ~7x cancellation and fp16 stats cost 1.6e-3 rel err (measured) vs
~1e-4 total with this split.
"""

import numpy as np

B = 4096
FD = 256
C = 32
D = 2
NSEG = C ** D          # 1024
NCORES = 8
RB = B // NCORES       # 512 rows per core
KT = RB // 128         # 4 k-chunks of 128 rows
EF = FD + 2            # 258: [mfn | m | m*sq]
ES = NSEG + 2 * D * C + 2   # 1154: [onehot | LQ | P | ones | E]
NMT = (ES + 127) // 128     # 10 m-tiles (last has 2 rows)

_compiled = {}


def _build_bass():
    from contextlib import ExitStack
    import concourse.bass as bass
    import concourse.bacc as bacc
    import concourse.tile as tile
    from concourse import mybir

    from concourse.tile import add_dep_helper

    f32 = mybir.dt.float32
    f16 = mybir.dt.float16
    Alu = mybir.AluOpType
    Act = mybir.ActivationFunctionType
    Ax = mybir.AxisListType

    nc = bacc.Bacc(num_devices=NCORES)

    feat = nc.dram_tensor("features", [RB, FD], f32, kind="ExternalInput")
    targ = nc.dram_tensor("targets", [RB, D * C], f32, kind="ExternalInput")
    maskf = nc.dram_tensor("maskf", [RB, 1], f32, kind="ExternalInput")
    # per-core XOR-permuted iota: iotaperm[p, j] = j ^ (128*core_id)
    iotap = nc.dram_tensor("iotaperm", [128, NSEG], f16, kind="ExternalInput")
    outd = nc.dram_tensor("out", [8], f32, kind="ExternalOutput")

    with ExitStack() as ctx:
        tc = ctx.enter_context(tile.TileContext(nc))
        consts = ctx.enter_context(tc.tile_pool(name="consts", bufs=1))
        work = ctx.enter_context(tc.tile_pool(name="work", bufs=1))
        keep = ctx.enter_context(tc.tile_pool(name="keep", bufs=1))
        psum = ctx.enter_context(tc.tile_pool(name="psum", bufs=1, space="PSUM"))
        dram = ctx.enter_context(tc.tile_pool(name="dram", bufs=1, space="DRAM"))

        # ---------------- constants ----------------
        ones128 = consts.tile([128, 1], f32)
        nc.vector.memset(ones128[:], 1.0)

        # ---- batched input loads spread over the queues ----
        tbig = keep.tile([128, KT, D * C], f32, name="tbig")
        nc.scalar.dma_start(
            out=tbig[:], in_=targ[:, :].rearrange("(a p) f -> p a f", p=128))
        mkbig = keep.tile([128, KT, 1], f32, name="mkbig")
        nc.scalar.dma_start(
            out=mkbig[:], in_=maskf[:, :].rearrange("(a p) f -> p a f", p=128))
        # two tiles (not halves of one) so chunk reads only wait their own DMA
        xbig0 = keep.tile([128, 2, FD], f32, name="xbig0")
        nc.sync.dma_start(
            out=xbig0[:],
            in_=feat[0:256, :].rearrange("(a p) f -> p a f", p=128))
        xbig1 = keep.tile([128, 2, FD], f32, name="xbig1")
        nc.gpsimd.dma_start(
            out=xbig1[:],
            in_=feat[256:512, :].rearrange("(a p) f -> p a f", p=128))
        iota1024 = consts.tile([128, NSEG], f16)
        nc.gpsimd.dma_start(out=iota1024[:], in_=iotap[:, :])

        def xchunk(kc):
            return xbig0[:, kc, :] if kc < 2 else xbig1[:, kc - 2, :]

        # biota[j] = 32 - j  (for first-argmax via reduce_max)
        biota = consts.tile([128, C], f32)
        nc.gpsimd.iota(biota[:], [[-1, C]], base=C, channel_multiplier=0,
                       allow_small_or_imprecise_dtypes=True)

        NST = 2 * D * C + 2   # 130 stats columns: [lq | p | ones | E]
        es_oh = [keep.tile([128, NSEG], f16, name=f"esoh_{kc}")
                 for kc in range(KT)]
        es_st = [keep.tile([128, NST], f16, name=f"esst_{kc}")
                 for kc in range(KT)]
        ef_16 = [keep.tile([128, EF], f16, name=f"eff_{kc}")
                 for kc in range(KT)]

        # ---- ACT phase 1: row sum-of-squares (Square table loads once) ----
        sqpack = keep.tile([128, KT], f32, name="sqpack")
        scrsq = keep.tile([128, FD], f32, name="scrsq")
        act_chain = []
        for kc in range(KT):
            act_chain.append(nc.scalar.activation(
                out=scrsq[:], in_=xchunk(kc), func=Act.Square,
                accum_out=sqpack[:, kc:kc + 1]))
        # ---- ACT phase 2: one Sqrt for all chunks ----
        normpack = keep.tile([128, KT], f32, name="normpack")
        act_chain.append(nc.scalar.sqrt(normpack[:], sqpack[:]))
        nc.vector.tensor_scalar_max(out=normpack[:], in0=normpack[:],
                                    scalar1=1e-12)
        invpack = keep.tile([128, KT], f32, name="invpack")
        nc.vector.reciprocal(invpack[:], normpack[:])
        # minv = m * inv  (fold mask into the normalization scale)
        minvpack = keep.tile([128, KT], f32, name="minvpack")
        nc.vector.tensor_tensor(out=minvpack[:], in0=invpack[:],
                                in1=mkbig[:, :, 0], op=Alu.mult)

        # ---- targets chains (DVE) + Ln (ACT phase 3) ----
        # es_st columns: [0:64 lq | 64:128 p | 128 ones | 129 E]
        # chunk-batched front: one add / one reduce / one reciprocal
        t1big = keep.tile([128, KT, D * C], f32, name="t1big")
        nc.vector.tensor_scalar_add(out=t1big[:], in0=tbig[:], scalar1=1e-10)
        invsb = keep.tile([128, KT * D], f32, name="invsb")
        nc.vector.reduce_sum(
            out=invsb[:],
            in_=t1big[:].rearrange("p a (d c) -> p (a d) c", c=C),
            axis=Ax.X)
        nc.vector.reciprocal(invsb[:], invsb[:])
        ln_acts = []
        for kc in range(KT):
            st_t = es_st[kc]
            pt = st_t[:, D * C:2 * D * C]
            lqt = st_t[:, 0:D * C]
            for d_ in range(D):
                nc.vector.tensor_scalar_mul(
                    out=pt[:, C * d_:C * (d_ + 1)],
                    in0=t1big[:, kc, C * d_:C * (d_ + 1)],
                    scalar1=invsb[:, kc * D + d_:kc * D + d_ + 1])
            ln_acts.append(nc.scalar.activation(out=lqt, in_=pt,
                                                func=Act.Ln))

            # ---- first-argmax per dim, then code = cls0 + 32*cls1 ----
            cls = work.tile([128, D], f32, name=f"cls_{kc}", tag=f"cl_{kc}")
            for d_ in range(D):
                pch = pt[:, C * d_:C * (d_ + 1)]
                mx = work.tile([128, 1], f32, name=f"mx_{kc}_{d_}",
                               tag=f"mx_{kc}_{d_}")
                nc.vector.reduce_max(out=mx[:], in_=pch, axis=Ax.X)
                cand = work.tile([128, C], f32, name=f"cand_{kc}_{d_}",
                                 tag=f"cd_{kc}_{d_}")
                # (p == max) * (32 - idx); reduce_max -> 32 - first_argmax
                nc.vector.scalar_tensor_tensor(
                    out=cand[:], in0=pch, scalar=mx[:], in1=biota[:],
                    op0=Alu.is_equal, op1=Alu.mult)
                mq = work.tile([128, 1], f32, name=f"mq_{kc}_{d_}",
                               tag=f"mq_{kc}_{d_}")
                nc.vector.reduce_max(out=mq[:], in_=cand[:], axis=Ax.X)
                nc.vector.tensor_scalar(
                    out=cls[:, d_:d_ + 1], in0=mq[:], scalar1=-1.0,
                    scalar2=float(C), op0=Alu.mult, op1=Alu.add)
            code = work.tile([128, 1], f32, name=f"code_{kc}", tag=f"co_{kc}")
            nc.vector.tensor_scalar(
                out=code[:], in0=cls[:, 1:2], scalar1=float(C),
                scalar2=cls[:, 0:1], op0=Alu.mult, op1=Alu.add)
            # ---- one-hot against the per-core permuted iota ----
            nc.vector.tensor_scalar(
                out=es_oh[kc][:], in0=iota1024[:], scalar1=code[:],
                scalar2=None, op0=Alu.is_equal)

        # ---- ext_feat = [x*(m*inv) | m | sq0*inv*minv] (ACT phase 4) ----
        copy_acts = []
        for kc in range(KT):
            ef_t = ef_16[kc]
            copy_acts.append(nc.scalar.activation(
                out=ef_t[:, 0:FD], in_=xchunk(kc), func=Act.Copy,
                scale=minvpack[:, kc:kc + 1]))
            nc.vector.tensor_copy(out=ef_t[:, FD:FD + 1], in_=mkbig[:, kc, :])
            nc.vector.tensor_scalar(out=ef_t[:, FD + 1:FD + 2],
                                    in0=sqpack[:, kc:kc + 1],
                                    scalar1=invpack[:, kc:kc + 1],
                                    scalar2=minvpack[:, kc:kc + 1],
                                    op0=Alu.mult, op1=Alu.mult)

        # E / ones columns, deferred: only the last two m-tiles need them
        for kc in range(KT):
            st_t = es_st[kc]
            scr64 = work.tile([128, D * C], f32, name=f"scr64_{kc}",
                              tag=f"s64_{kc}")
            nc.vector.tensor_tensor(out=scr64[:],
                                    in0=st_t[:, D * C:2 * D * C],
                                    in1=st_t[:, 0:D * C], op=Alu.mult)
            escr = work.tile([128, 1], f32, name=f"escr_{kc}",
                             tag=f"es_{kc}")
            nc.vector.reduce_sum(out=escr[:], in_=scr64[:], axis=Ax.X)
            nc.vector.tensor_copy(out=st_t[:, NST - 1:NST], in_=escr[:])
            nc.vector.memset(st_t[:, NST - 2:NST - 1], 1.0)

        # keep ACT ops grouped by function (avoid act-table reload thrash);
        # table-less Copies run before the Lns so ef is ready sooner
        act_chain = act_chain + copy_acts + ln_acts
        for a, b in zip(act_chain[1:], act_chain[:-1]):
            add_dep_helper(a.ins, b.ins, sync=False,
                           reason="act table grouping")

        # ---------------- payload tiles ----------------
        # seg_pay slot b = m-tile b = permuted segment block (true block b^c)
        seg_pay = keep.tile([128, 8, EF], f16, name="seg_pay")
        # stats: slot 0 = m-tile 8 ([LQ|P]), slot 1 p0/p1 = F/E rows, fp32
        st_pay = keep.tile([128, 2, EF], f32, name="st_pay")
        nc.vector.memset(st_pay[:, 1:2, :], 0.0)
        inb_st = dram.tile([128, 2, EF], f32, name="inb_st")
        outb_st = dram.tile([128, 2, EF], f32, name="outb_st",
                            addr_space="Shared")

        # ---------------- the one big matmul ----------------
        for mt in range(NMT):
            mlo = mt * 128
            msz = min(128, ES - mlo)
            ps = psum.tile([msz, EF], f32, name=f"ps_{mt}", tag=f"ps_{mt % 7}")
            for kc in range(KT):
                if mt < 8:
                    lhsT = es_oh[kc][:, mlo:mlo + msz]
                else:
                    lhsT = es_st[kc][:, mlo - NSEG:mlo - NSEG + msz]
                nc.tensor.matmul(out=ps[:], lhsT=lhsT, rhs=ef_16[kc][:],
                                 start=(kc == 0), stop=(kc == KT - 1))
            if mt < 8:
                # alternate engines so copies keep pace with the matmuls
                # (gpsimd cannot read PSUM; scalar's Copy is table-less)
                if mt % 2 == 0:
                    nc.vector.tensor_copy(out=seg_pay[:, mt, :], in_=ps[:])
                else:
                    nc.scalar.activation(out=seg_pay[:, mt, :], in_=ps[:],
                                         func=Act.Copy)
            elif mt == 8:
                nc.vector.tensor_copy(out=st_pay[:, 0, :], in_=ps[:])
            else:
                nc.vector.tensor_copy(out=st_pay[0:2, 1, :], in_=ps[0:2, :])

        # ---------------- stats AllReduce (also the entry barrier) --------
        nc.sync.dma_start(out=inb_st[:], in_=st_pay[:])
        ar_op = nc.gpsimd.collective_compute(
            "AllReduce", mybir.AluOpType.add,
            replica_groups=[list(range(NCORES))],
            ins=[inb_st.opt()], outs=[outb_st.opt()])

        # ---------------- segment XOR reduce-scatter ----------------------
        # round r: send my slot r (= true block (c^r)'s partial) to core c^r.
        rsegs = [keep.tile([128, EF], f16, name=f"rseg_{r}")
                 for r in range(1, NCORES)]
        lsem = nc.alloc_semaphore("xch_local")
        seg_sems = [nc.alloc_semaphore(f"seg_arrive_{r}")
                    for r in range(1, NCORES)]
        z_sems = [nc.alloc_semaphore(f"z_arrive_{r}")
                  for r in range(1, NCORES)]
        for r in range(1, NCORES):
            rdests = [None] * NCORES
            rdests[r] = (0, r)
            nc.gpsimd.remote_dma_broadcast(
                out_ap=rsegs[r - 1][:], in_ap=seg_pay[:, r, :],
                remote_sem=seg_sems[r - 1], local_sem=lsem, rdests=rdests)
        # stats loads issued here so the exchange trigger can gate on a true
        # data consumer of the AllReduce result (guarantees AR completion =
        # every peer past its preamble, before any remote write fires)
        ut = keep.tile([64, EF], f32, name="ut")
        ut_load = nc.sync.dma_start(out=ut[:], in_=outb_st[0:64, 0, :])
        vt = keep.tile([64, EF], f32, name="vt")
        nc.sync.dma_start(out=vt[:], in_=outb_st[64:128, 0, :])
        frow = keep.tile([1, EF], f32, name="frow")
        nc.scalar.dma_start(out=frow[:], in_=outb_st[0:1, 1, :])
        erow = keep.tile([1, EF], f32, name="erow")
        nc.scalar.dma_start(out=erow[:], in_=outb_st[1:2, 1, :])

        # a compute probe that READS the AR result: completes only after the
        # collective's data landed in SBUF (DMA-completion sem), so gating
        # the trigger on it guarantees every peer passed its preamble.
        probe = keep.tile([1, 1], f32, name="probe")
        probe_op = nc.vector.tensor_copy(out=probe[:], in_=ut[0:1, 0:1])
        trig1 = nc.gpsimd.trigger_dma(count=None)
        add_dep_helper(trig1.ins, ar_op.ins, sync=True,
                       reason="AR completion is the entry barrier")
        add_dep_helper(trig1.ins, probe_op.ins, sync=True,
                       reason="AR data-consumer gate")

        # ---------------- owned-block reduction + center term -------------
        # arrival waits attached post-scheduling (the single-core scheduling
        # sim cannot model peer-driven semaphore increments). Every gated op
        # ALSO gets a scheduler-visible ordering dep on the AR probe: without
        # it, Tile places these adds early in the DVE stream, ahead of the
        # st_pay copies feeding the AllReduce — at runtime the adds block the
        # DVE queue waiting for arrivals, arrivals need every peer's sends,
        # sends need their AllReduce, which needs their (blocked) DVE: a
        # cross-core deadlock (caught by MultiCoreSim).
        post_waits = []

        def gated(op, *sems):
            add_dep_helper(op.ins, probe_op.ins, sync=False,
                           reason="schedule after AR feed-chain")
            for sem in sems:
                post_waits.append((op, sem))
            return op

        acc = keep.tile([128, EF], f16, name="acc")
        b01 = keep.tile([128, EF], f16, name="b01")
        b23 = keep.tile([128, EF], f16, name="b23")
        b45 = keep.tile([128, EF], f16, name="b45")
        gated(nc.vector.tensor_tensor(
            out=b01[:], in0=seg_pay[:, 0, :], in1=rsegs[0][:],
            op=Alu.add), seg_sems[0])
        gated(nc.vector.tensor_tensor(
            out=b23[:], in0=rsegs[1][:], in1=rsegs[2][:],
            op=Alu.add), seg_sems[1], seg_sems[2])
        gated(nc.vector.tensor_tensor(
            out=b45[:], in0=rsegs[3][:], in1=rsegs[4][:],
            op=Alu.add), seg_sems[3], seg_sems[4])
        nc.vector.tensor_tensor(out=b01[:], in0=b01[:], in1=b23[:],
                                op=Alu.add)
        gated(nc.vector.tensor_tensor(
            out=b23[:], in0=rsegs[5][:], in1=rsegs[6][:],
            op=Alu.add), seg_sems[5], seg_sems[6])
        nc.vector.tensor_tensor(out=b45[:], in0=b45[:], in1=b23[:],
                                op=Alu.add)
        nc.vector.tensor_tensor(out=acc[:], in0=b01[:], in1=b45[:],
                                op=Alu.add)

        # per-partition center-term partial: ||sum||^2 / max(cnt,1)
        scrA = keep.tile([128, FD], f32, name="scrA")
        nc.vector.tensor_tensor(out=scrA[:], in0=acc[:, 0:FD],
                                in1=acc[:, 0:FD], op=Alu.mult)
        # padded to 64 fp32 per partition so the remote descriptor moves
        # 256-byte lines (a 4-byte/partition transfer is degenerate)
        spay = keep.tile([128, 64], f32, name="spay")
        nc.vector.memset(spay[:, 1:64], 0.0)
        nrm1 = keep.tile([128, 1], f32, name="nrm1")
        nc.vector.reduce_sum(out=nrm1[:], in_=scrA[:], axis=Ax.X)
        cnt1 = keep.tile([128, 1], f32, name="cnt1")
        nc.vector.tensor_scalar_max(out=cnt1[:], in0=acc[:, FD:FD + 1],
                                    scalar1=1.0)
        nc.vector.reciprocal(cnt1[:], cnt1[:])
        nc.vector.tensor_tensor(out=spay[:, 0:1], in0=nrm1[:], in1=cnt1[:],
                                op=Alu.mult)

        # ---------------- scalar-partial XOR broadcast --------------------
        rzs = [keep.tile([128, 64], f32, name=f"rz_{r}")
               for r in range(1, NCORES)]
        for r in range(1, NCORES):
            rdests = [None] * NCORES
            rdests[r] = (0, r)
            nc.gpsimd.remote_dma_broadcast(
                out_ap=rzs[r - 1][:], in_ap=spay[:],
                remote_sem=z_sems[r - 1], local_sem=lsem, rdests=rdests)
        nc.gpsimd.trigger_dma(count=None)

        Z = keep.tile([128, 8], f32, name="Z")
        nc.vector.memset(Z[:], 0.0)
        z01 = keep.tile([128, 1], f32, name="z01")
        z23 = keep.tile([128, 1], f32, name="z23")
        z45 = keep.tile([128, 1], f32, name="z45")
        gated(nc.vector.tensor_tensor(
            out=z01[:], in0=spay[:, 0:1], in1=rzs[0][:, 0:1],
            op=Alu.add), z_sems[0])
        gated(nc.vector.tensor_tensor(
            out=z23[:], in0=rzs[1][:, 0:1], in1=rzs[2][:, 0:1],
            op=Alu.add), z_sems[1], z_sems[2])
        gated(nc.vector.tensor_tensor(
            out=z45[:], in0=rzs[3][:, 0:1], in1=rzs[4][:, 0:1],
            op=Alu.add), z_sems[3], z_sems[4])
        nc.vector.tensor_tensor(out=z01[:], in0=z01[:], in1=z23[:],
                                op=Alu.add)
        gated(nc.vector.tensor_tensor(
            out=z23[:], in0=rzs[5][:, 0:1], in1=rzs[6][:, 0:1],
            op=Alu.add), z_sems[5], z_sems[6])
        nc.vector.tensor_tensor(out=z45[:], in0=z45[:], in1=z23[:],
                                op=Alu.add)
        nc.vector.tensor_tensor(out=Z[:, 0:1], in0=z01[:], in1=z45[:],
                                op=Alu.add)

        # ---------------- stats epilogue (from the AllReduce) -------------
        scrU = keep.tile([64, FD], f32, name="scrU")
        nc.vector.tensor_tensor(out=scrU[:], in0=ut[:, 0:FD],
                                in1=vt[:, 0:FD], op=Alu.mult)
        nc.vector.reduce_sum(out=Z[0:64, 1:2], in_=scrU[:], axis=Ax.X)
        nc.vector.tensor_tensor(out=Z[0:64, 2:3], in0=vt[:, FD + 1:FD + 2],
                                in1=ut[:, FD:FD + 1], op=Alu.mult)     # Psq*L
        nc.vector.tensor_tensor(out=Z[0:64, 3:4], in0=vt[:, FD:FD + 1],
                                in1=ut[:, FD + 1:FD + 2], op=Alu.mult)  # Pbar*Lsq
        scrF = keep.tile([1, FD], f32, name="scrF")
        nc.vector.tensor_tensor(out=scrF[:], in0=frow[0:1, 0:FD],
                                in1=erow[0:1, 0:FD], op=Alu.mult)
        nc.vector.reduce_sum(out=Z[0:1, 4:5], in_=scrF[:], axis=Ax.X)  # Fe.F

        zred = psum.tile([1, 8], f32, name="zred", tag="ps_0")
        nc.tensor.matmul(out=zred[:], lhsT=ones128[:], rhs=Z[:],
                         start=True, stop=True)
        zs = keep.tile([1, 8], f32, name="zs")
        nc.vector.tensor_copy(out=zs[:], in_=zred[:])

        # scalars: M=F[256], a=F[257], e=E[256], se=E[257] (all fp32)
        Mv = frow[0:1, FD:FD + 1]
        av = frow[0:1, FD + 1:FD + 2]
        ev = erow[0:1, FD:FD + 1]
        sev = erow[0:1, FD + 1:FD + 2]
        s_center = zs[0:1, 0:1]
        uv = zs[0:1, 1:2]
        psql = zs[0:1, 2:3]
        pbarlsq = zs[0:1, 3:4]
        fef = zs[0:1, 4:5]

        fin = keep.tile([1, 16], f32, name="fin")
        t_ = lambda i: fin[0:1, i:i + 1]
        # f0 = se*M ; f1 = a*e ; f2 = f0+f1
        nc.vector.tensor_tensor(out=t_(8), in0=sev, in1=Mv, op=Alu.mult)
        nc.vector.tensor_tensor(out=t_(9), in0=av, in1=ev, op=Alu.mult)
        nc.vector.tensor_tensor(out=t_(10), in0=t_(8), in1=t_(9), op=Alu.add)
        # f3 = -2*fef + f2
        nc.vector.tensor_scalar(out=t_(11), in0=fef, scalar1=-2.0,
                                scalar2=t_(10), op0=Alu.mult, op1=Alu.add)
        # f4 = f3 - psql ; f5 = f4 - pbarlsq
        nc.vector.tensor_tensor(out=t_(12), in0=t_(11), in1=psql, op=Alu.subtract)
        nc.vector.tensor_tensor(out=t_(13), in0=t_(12), in1=pbarlsq, op=Alu.subtract)
        # SD = 2*uv + f5
        nc.vector.tensor_scalar(out=t_(14), in0=uv, scalar1=2.0,
                                scalar2=t_(13), op0=Alu.mult, op1=Alu.add)
        # md = M*(M-1) ; rmd = 1/md ; div = SD*rmd*(-1/D)
        nc.vector.tensor_scalar(out=t_(15), in0=Mv, scalar1=-1.0,
                                scalar2=Mv, op0=Alu.add, op1=Alu.mult)
        nc.vector.reciprocal(t_(15), t_(15))
        nc.vector.tensor_tensor(out=t_(1), in0=t_(14), in1=t_(15), op=Alu.mult)
        nc.vector.tensor_scalar_mul(out=t_(1), in0=t_(1), scalar1=-1.0 / D)
        # tight = (a - s_center)/M
        nc.vector.tensor_tensor(out=t_(7), in0=av, in1=s_center, op=Alu.subtract)
        nc.vector.reciprocal(t_(6), Mv)
        nc.vector.tensor_tensor(out=t_(2), in0=t_(7), in1=t_(6), op=Alu.mult)
        # total = 0.1*div + 0.1*tight
        nc.vector.tensor_tensor(out=t_(0), in0=t_(1), in1=t_(2), op=Alu.add)
        nc.vector.tensor_scalar_mul(out=t_(0), in0=t_(0), scalar1=0.1)
        # debug slots
        nc.vector.tensor_copy(out=t_(3), in_=Mv)
        nc.vector.tensor_copy(out=t_(4), in_=av)
        nc.vector.tensor_copy(out=t_(5), in_=sev)

        nc.sync.dma_start(out=outd[None, :], in_=fin[0:1, 0:8])

    # TileContext has exited (scheduling sim done) — attach the runtime
    # arrival gates the sim could not model: each first read of a peer
    # buffer waits for that round's remote-semaphore (+2 per broadcast).
    for op, sem in post_waits:
        op.wait_op(sem, 2, "sem-ge", check=False)

    nc.finalize()
    return nc


def _get_compiled():
    if "nc" not in _compiled:
        _compiled["nc"] = _build_bass()
    return _compiled["nc"]


_IOTAPERMS = [
    np.tile((np.arange(NSEG) ^ (128 * c)).astype(np.float16), (128, 1))
    for c in range(NCORES)
]


def _make_in_maps(features, targets, mask):
    features = np.ascontiguousarray(np.asarray(features, dtype=np.float32))
    targets = np.ascontiguousarray(np.asarray(targets, dtype=np.float32))
    maskf = np.asarray(mask).astype(np.float32).reshape(B, 1)
    in_maps = []
    for i in range(NCORES):
        sl = slice(i * RB, (i + 1) * RB)
        in_maps.append({
            "features": features[sl],
            "targets": targets[sl],
            "maskf": np.ascontiguousarray(maskf[sl]),
            "iotaperm": _IOTAPERMS[i],
        })
    return in_maps


def kernel(features, targets, mask):
    from concourse.bass_utils import run_bass_kernel_spmd

    nc = _get_compiled()
    in_maps = _make_in_maps(features, targets, mask)
    res = run_bass_kernel_spmd(nc, in_maps, list(range(NCORES)))
    out = res.results[0]["out"]
    total = np.float32(out[0])
    diversity = np.float32(out[1])
    tightness = np.float32(out[2])
    return total, diversity, tightness


# revision 31
# speedup vs baseline: 1.8277x; 1.6511x over previous
"""Trainium2 Bass kernel for CategoricalEntropyRegLoss.

Math: both loss terms factor so the [B,B] pairwise matrices are never built.

  feat_dists = sq_j + sq_k - 2 fn_j.fn_k            (rank FD+2)
  target_dists = (E_j - P_j.LQ_k) / D               (rank DC+1)
  S = sum_{jk} m_j m_k feat_dists * target_dists    (diag is exactly 0)
    = [ se*M + a*e - 2 Fe.F - Psq.L - Pbar.Lsq + 2 <U,V> ] / D
  tightness*M = a - sum_s ||seg_sum_s||^2 / max(cnt_s,1)

Everything needed is one matmul per core:
  out[1154, 258] = ext_seg^T @ ext_feat
  ext_seg  = [ onehot(code) | LQ | P | 1 | E ]      (B x 1154)
  ext_feat = [ m*fn | m | m*sq ]                    (B x 258)

Cross-core reduction: TWO pipelined NRT AllReduces. The stats rows
(130 x 258 fp32, 264KB) are ready ~10us before the segment one-hot
matmuls finish, so their AllReduce triggers early and absorbs the
one-time collective bring-up (~14-30us) while the segment matmuls run.
The segment AllReduce (1024 x 258 fp16, 528KB) queues right behind it;
a second collective starts ~1us after the first's mesh ends (measured).
The stats epilogue overlaps the second mesh.

(Alternatives measured and rejected: a remote-DMA SBUF exchange — each
blocking event-semaphore wait costs ~11-14us and a full-payload
allgather hits the ~45GB/s per-core DMA ceiling; a single combined
AllReduce — serializes the collective behind the last matmul and forces
one payload dtype.)

Front-end ordering: Ln table preloads during input DMA and the Lns run
first (stats path is the critical path); row sum-of-squares moves to
the otherwise-idle gpsimd; 1/norm uses one ACT Rsqrt; the argmax chain
is chunk-batched on DVE.

Precision: matmul operands fp16 (one-hot exact, 2x PE rate, PSUM fp32).
Segment rows travel fp16 (feeds only the squared-norm center term,
errors average over 1024 segments); stats travel fp32 (the diversity
total has ~7x cancellation; fp16 stats cost 1.6e-3 rel err, fp32 ~1e-5).
"""

import numpy as np

B = 4096
FD = 256
C = 32
D = 2
NSEG = C ** D          # 1024
NCORES = 8
RB = B // NCORES       # 512 rows per core
KT = RB // 128         # 4 k-chunks of 128 rows
EF = FD + 2            # 258: [mfn | m | m*sq]
ES = NSEG + 2 * D * C + 2   # 1154: [onehot | LQ | P | ones | E]
NMT = (ES + 127) // 128     # 10 m-tiles (last has 2 rows)

_compiled = {}


def _build_bass():
    from contextlib import ExitStack
    import concourse.bass as bass
    import concourse.bacc as bacc
    import concourse.tile as tile
    from concourse import mybir

    from concourse.tile import add_dep_helper

    f32 = mybir.dt.float32
    f16 = mybir.dt.float16
    Alu = mybir.AluOpType
    Act = mybir.ActivationFunctionType
    Ax = mybir.AxisListType

    nc = bacc.Bacc(num_devices=NCORES)

    feat = nc.dram_tensor("features", [RB, FD], f32, kind="ExternalInput")
    targ = nc.dram_tensor("targets", [RB, D * C], f32, kind="ExternalInput")
    maskf = nc.dram_tensor("maskf", [RB, 1], f32, kind="ExternalInput")
    # plain iota row, replicated to 128 partitions (cheaper than IOTA op)
    iotap = nc.dram_tensor("iotaperm", [128, NSEG], f16, kind="ExternalInput")
    outd = nc.dram_tensor("out", [8], f32, kind="ExternalOutput")

    with ExitStack() as ctx:
        tc = ctx.enter_context(tile.TileContext(nc))
        consts = ctx.enter_context(tc.tile_pool(name="consts", bufs=1))
        work = ctx.enter_context(tc.tile_pool(name="work", bufs=1))
        keep = ctx.enter_context(tc.tile_pool(name="keep", bufs=1))
        psum = ctx.enter_context(tc.tile_pool(name="psum", bufs=1, space="PSUM"))
        dram = ctx.enter_context(tc.tile_pool(name="dram", bufs=1, space="DRAM"))

        # ---------------- constants ----------------
        ones128 = consts.tile([128, 1], f32)
        nc.vector.memset(ones128[:], 1.0)

        # ---- batched input loads spread over the queues ----
        tbig = keep.tile([128, KT, D * C], f32, name="tbig")
        nc.scalar.dma_start(
            out=tbig[:], in_=targ[:, :].rearrange("(a p) f -> p a f", p=128))
        mkbig = keep.tile([128, KT, 1], f32, name="mkbig")
        nc.scalar.dma_start(
            out=mkbig[:], in_=maskf[:, :].rearrange("(a p) f -> p a f", p=128))
        # two tiles (not halves of one) so chunk reads only wait their own DMA
        xbig0 = keep.tile([128, 2, FD], f32, name="xbig0")
        nc.sync.dma_start(
            out=xbig0[:],
            in_=feat[0:256, :].rearrange("(a p) f -> p a f", p=128))
        xbig1 = keep.tile([128, 2, FD], f32, name="xbig1")
        nc.gpsimd.dma_start(
            out=xbig1[:],
            in_=feat[256:512, :].rearrange("(a p) f -> p a f", p=128))
        iota1024 = consts.tile([128, NSEG], f16)
        nc.sync.dma_start(out=iota1024[:], in_=iotap[:, :])

        def xchunk(kc):
            return xbig0[:, kc, :] if kc < 2 else xbig1[:, kc - 2, :]

        # biota[j] = 32 - j  (for first-argmax via reduce_max)
        biota = consts.tile([128, C], f32)
        nc.gpsimd.iota(biota[:], [[-1, C]], base=C, channel_multiplier=0,
                       allow_small_or_imprecise_dtypes=True)

        NST = 2 * D * C + 2   # 130 stats columns: [lq | p | ones | E]
        es_oh = [keep.tile([128, NSEG], f16, name=f"esoh_{kc}")
                 for kc in range(KT)]
        es_st = [keep.tile([128, NST], f16, name=f"esst_{kc}")
                 for kc in range(KT)]
        ef_16 = [keep.tile([128, EF], f16, name=f"eff_{kc}")
                 for kc in range(KT)]

        # ---- row sum-of-squares: squares on gpsimd (otherwise idle; keeps
        # ACT free for the Ln-first ordering), one batched DVE reduce
        # (gpsimd cannot reduce along the free axis) ----
        sqpack = keep.tile([128, KT], f32, name="sqpack")
        scrg4 = keep.tile([128, KT, FD], f32, name="scrg4")
        for kc in range(KT):
            nc.gpsimd.tensor_tensor(out=scrg4[:, kc, :], in0=xchunk(kc),
                                    in1=xchunk(kc), op=Alu.mult)
        nc.vector.reduce_sum(out=sqpack[:], in_=scrg4[:], axis=Ax.X)

        # ---- targets chains (DVE) ----
        # es_st columns: [0:64 lq | 64:128 p | 128 ones | 129 E]
        t1big = keep.tile([128, KT, D * C], f32, name="t1big")
        nc.vector.tensor_scalar_add(out=t1big[:], in0=tbig[:], scalar1=1e-10)
        invsb = keep.tile([128, KT * D], f32, name="invsb")
        nc.vector.reduce_sum(
            out=invsb[:],
            in_=t1big[:].rearrange("p a (d c) -> p (a d) c", c=C),
            axis=Ax.X)
        nc.vector.reciprocal(invsb[:], invsb[:])

        # ACT phase 1: Ln table preload (dummy) then the 4 Lns — the stats
        # m-tiles are the critical path (they feed the early AllReduce)
        lnscr = work.tile([128, 1], f32, name="lnscr", tag="lnscr")
        act_chain = [nc.scalar.activation(out=lnscr[:], in_=ones128[:],
                                          func=Act.Ln)]
        ln_acts = []
        for kc in range(KT):
            st_t = es_st[kc]
            pt = st_t[:, D * C:2 * D * C]
            for d_ in range(D):
                nc.vector.tensor_scalar_mul(
                    out=pt[:, C * d_:C * (d_ + 1)],
                    in0=t1big[:, kc, C * d_:C * (d_ + 1)],
                    scalar1=invsb[:, kc * D + d_:kc * D + d_ + 1])
            ln_acts.append(nc.scalar.activation(
                out=st_t[:, 0:D * C], in_=pt, func=Act.Ln))

        # E / ones columns right after each chunk's Ln
        for kc in range(KT):
            st_t = es_st[kc]
            scr64 = work.tile([128, D * C], f32, name=f"scr64_{kc}",
                              tag=f"s64_{kc}")
            nc.vector.tensor_tensor(out=scr64[:],
                                    in0=st_t[:, D * C:2 * D * C],
                                    in1=st_t[:, 0:D * C], op=Alu.mult)
            escr = work.tile([128, 1], f32, name=f"escr_{kc}",
                             tag=f"es_{kc}")
            nc.vector.reduce_sum(out=escr[:], in_=scr64[:], axis=Ax.X)
            nc.vector.tensor_copy(out=st_t[:, NST - 1:NST], in_=escr[:])
            nc.vector.memset(st_t[:, NST - 2:NST - 1], 1.0)

        # ---- 1/norm: one ACT Sqrt + DVE reciprocal (phase 2) ----
        nc.vector.tensor_scalar_max(out=sqpack[:], in0=sqpack[:],
                                    scalar1=1e-24)
        normpack = keep.tile([128, KT], f32, name="normpack")
        act_chain.append(nc.scalar.sqrt(normpack[:], sqpack[:]))
        invpack = keep.tile([128, KT], f32, name="invpack")
        nc.vector.reciprocal(invpack[:], normpack[:])
        minvpack = keep.tile([128, KT], f32, name="minvpack")
        nc.vector.tensor_tensor(out=minvpack[:], in0=invpack[:],
                                in1=mkbig[:, :, 0], op=Alu.mult)

        # ---- ext_feat = [x*(m*inv) | m | sq*inv*minv] (ACT phase 3) ----
        copy_acts = []
        for kc in range(KT):
            ef_t = ef_16[kc]
            copy_acts.append(nc.scalar.activation(
                out=ef_t[:, 0:FD], in_=xchunk(kc), func=Act.Copy,
                scale=minvpack[:, kc:kc + 1]))
            nc.vector.tensor_copy(out=ef_t[:, FD:FD + 1], in_=mkbig[:, kc, :])
            nc.vector.tensor_scalar(out=ef_t[:, FD + 1:FD + 2],
                                    in0=sqpack[:, kc:kc + 1],
                                    scalar1=invpack[:, kc:kc + 1],
                                    scalar2=minvpack[:, kc:kc + 1],
                                    op0=Alu.mult, op1=Alu.mult)

        # ---- chunk-batched first-argmax, then code = cls0 + 32*cls1 ----
        AD = KT * D   # 8 (kc, d) groups
        mx8 = work.tile([128, AD], f32, name="mx8", tag="mx8")
        nc.vector.reduce_max(
            out=mx8[:],
            in_=t1big[:].rearrange("p a (d c) -> p (a d) c", c=C),
            axis=Ax.X)
        cand8 = work.tile([128, AD, C], f32, name="cand8", tag="cand8")
        for kc in range(KT):
            for d_ in range(D):
                g = kc * D + d_
                # (t1 == max) * (32 - idx); reduce_max -> 32 - first_argmax
                nc.vector.scalar_tensor_tensor(
                    out=cand8[:, g, :],
                    in0=t1big[:, kc, C * d_:C * (d_ + 1)],
                    scalar=mx8[:, g:g + 1], in1=biota[:],
                    op0=Alu.is_equal, op1=Alu.mult)
        mq8 = work.tile([128, AD], f32, name="mq8", tag="mq8")
        nc.vector.reduce_max(out=mq8[:], in_=cand8[:], axis=Ax.X)
        cls8 = work.tile([128, AD], f32, name="cls8", tag="cls8")
        nc.vector.tensor_scalar(out=cls8[:], in0=mq8[:], scalar1=-1.0,
                                scalar2=float(C), op0=Alu.mult, op1=Alu.add)
        # code4[kc] = cls[kc,0] + 32*cls[kc,1]
        code4 = work.tile([128, KT], f32, name="code4", tag="code4")
        cls_v = cls8[:].rearrange("p (a two) -> p a two", two=2)
        nc.vector.tensor_scalar_mul(out=code4[:], in0=cls_v[:, :, 1],
                                    scalar1=float(C))
        nc.vector.tensor_tensor(out=code4[:], in0=code4[:],
                                in1=cls_v[:, :, 0], op=Alu.add)
        for kc in range(KT):
            nc.vector.tensor_scalar(
                out=es_oh[kc][:], in0=iota1024[:],
                scalar1=code4[:, kc:kc + 1],
                scalar2=None, op0=Alu.is_equal)

        # keep ACT ops grouped by function (avoid act-table reload thrash)
        act_chain = (act_chain[:1] + ln_acts + act_chain[1:] + copy_acts)
        for a, b in zip(act_chain[1:], act_chain[:-1]):
            add_dep_helper(a.ins, b.ins, sync=False,
                           reason="act table grouping")

        # ---------------- payload tiles + AllReduce buffers ----------------
        seg_pay = keep.tile([128, 8, EF], f16, name="seg_pay")
        st_pay = keep.tile([128, 2, EF], f32, name="st_pay")
        nc.vector.memset(st_pay[:, 1:2, :], 0.0)
        inb_st = dram.tile([128, 2, EF], f32, name="inb_st")
        outb_st = dram.tile([128, 2, EF], f32, name="outb_st",
                            addr_space="Shared")
        inb_seg = dram.tile([128, 8, EF], f16, name="inb_seg")
        outb_seg = dram.tile([128, 8, EF], f16, name="outb_seg",
                             addr_space="Shared")

        # ---------------- matmuls: stats tiles first ----------------------
        # mt 8/9 feed the early stats AllReduce; the 8 one-hot tiles follow.
        for mt in [8, 9] + list(range(8)):
            mlo = mt * 128
            msz = min(128, ES - mlo)
            ps = psum.tile([msz, EF], f32, name=f"ps_{mt}", tag=f"ps_{mt % 7}")
            for kc in range(KT):
                if mt < 8:
                    lhsT = es_oh[kc][:, mlo:mlo + msz]
                else:
                    lhsT = es_st[kc][:, mlo - NSEG:mlo - NSEG + msz]
                nc.tensor.matmul(out=ps[:], lhsT=lhsT, rhs=ef_16[kc][:],
                                 start=(kc == 0), stop=(kc == KT - 1))
            if mt < 8:
                # alternate engines so copies keep pace with the matmuls
                # (gpsimd cannot read PSUM; scalar's Copy is table-less)
                if mt % 2 == 0:
                    nc.vector.tensor_copy(out=seg_pay[:, mt, :], in_=ps[:])
                else:
                    nc.scalar.activation(out=seg_pay[:, mt, :], in_=ps[:],
                                         func=Act.Copy)
            elif mt == 8:
                nc.vector.tensor_copy(out=st_pay[:, 0, :], in_=ps[:])
            else:
                nc.vector.tensor_copy(out=st_pay[0:2, 1, :], in_=ps[0:2, :])
                # stats payload complete -> store + AllReduce immediately
                nc.sync.dma_start(out=inb_st[:], in_=st_pay[:])
                nc.gpsimd.collective_compute(
                    "AllReduce", mybir.AluOpType.add,
                    replica_groups=[list(range(NCORES))],
                    ins=[inb_st.opt()], outs=[outb_st.opt()])

        # segment payload store + second AllReduce (queues behind the first)
        nc.sync.dma_start(out=inb_seg[:], in_=seg_pay[:])
        nc.gpsimd.collective_compute(
            "AllReduce", mybir.AluOpType.add,
            replica_groups=[list(range(NCORES))],
            ins=[inb_seg.opt()], outs=[outb_seg.opt()])

        # ---------------- stats epilogue (overlaps the segment mesh) ------
        ut = keep.tile([64, EF], f32, name="ut")
        nc.sync.dma_start(out=ut[:], in_=outb_st[0:64, 0, :])
        vt = keep.tile([64, EF], f32, name="vt")
        nc.sync.dma_start(out=vt[:], in_=outb_st[64:128, 0, :])
        frow = keep.tile([1, EF], f32, name="frow")
        nc.scalar.dma_start(out=frow[:], in_=outb_st[0:1, 1, :])
        erow = keep.tile([1, EF], f32, name="erow")
        nc.scalar.dma_start(out=erow[:], in_=outb_st[1:2, 1, :])

        Z = keep.tile([128, 8], f32, name="Z")
        nc.vector.memset(Z[:], 0.0)
        scrU = keep.tile([64, FD], f32, name="scrU")
        nc.vector.tensor_tensor(out=scrU[:], in0=ut[:, 0:FD],
                                in1=vt[:, 0:FD], op=Alu.mult)
        nc.vector.reduce_sum(out=Z[0:64, 1:2], in_=scrU[:], axis=Ax.X)
        nc.vector.tensor_tensor(out=Z[0:64, 2:3], in0=vt[:, FD + 1:FD + 2],
                                in1=ut[:, FD:FD + 1], op=Alu.mult)     # Psq*L
        nc.vector.tensor_tensor(out=Z[0:64, 3:4], in0=vt[:, FD:FD + 1],
                                in1=ut[:, FD + 1:FD + 2], op=Alu.mult)  # Pbar*Lsq
        scrF = keep.tile([1, FD], f32, name="scrF")
        nc.vector.tensor_tensor(out=scrF[:], in0=frow[0:1, 0:FD],
                                in1=erow[0:1, 0:FD], op=Alu.mult)
        nc.vector.reduce_sum(out=Z[0:1, 4:5], in_=scrF[:], axis=Ax.X)  # Fe.F

        # ---------------- segment epilogue ----------------
        # loads split over two queues; squares split ACT/DVE
        big0 = keep.tile([128, 4, EF], f16, name="big0")
        nc.sync.dma_start(out=big0[:], in_=outb_seg[:, 0:4, :])
        big1 = keep.tile([128, 4, EF], f16, name="big1")
        nc.scalar.dma_start(out=big1[:], in_=outb_seg[:, 4:8, :])

        nrmp = keep.tile([128, 8], f32, name="nrmp")
        sq_acts = []
        for s in range(4):
            sq_acts.append(nc.scalar.activation(
                out=scrg4[:, 0, :], in_=big1[:, s, 0:FD], func=Act.Square,
                accum_out=nrmp[:, 4 + s:5 + s]))
        for a, b in zip(sq_acts[1:], sq_acts[:-1]):
            add_dep_helper(a.ins, b.ins, sync=False, reason="act grouping")
        scrB = keep.tile([128, 4, FD], f32, name="scrB")
        nc.vector.tensor_tensor(out=scrB[:], in0=big0[:, :, 0:FD],
                                in1=big0[:, :, 0:FD], op=Alu.mult)
        nc.vector.reduce_sum(out=nrmp[:, 0:4], in_=scrB[:], axis=Ax.X)
        cdp = keep.tile([128, 8], f32, name="cdp")
        nc.vector.tensor_scalar_max(out=cdp[:, 0:4], in0=big0[:, :, FD],
                                    scalar1=1.0)
        nc.vector.tensor_scalar_max(out=cdp[:, 4:8], in0=big1[:, :, FD],
                                    scalar1=1.0)
        rcdp = keep.tile([128, 8], f32, name="rcdp")
        nc.vector.reciprocal(rcdp[:], cdp[:])
        termp = keep.tile([128, 8], f32, name="termp")
        nc.vector.tensor_tensor(out=termp[:], in0=nrmp[:], in1=rcdp[:],
                                op=Alu.mult)
        nc.vector.reduce_sum(out=Z[:, 0:1], in_=termp[:], axis=Ax.X)

        zred = psum.tile([1, 8], f32, name="zred", tag="ps_0")
        nc.tensor.matmul(out=zred[:], lhsT=ones128[:], rhs=Z[:],
                         start=True, stop=True)
        zs = keep.tile([1, 8], f32, name="zs")
        nc.vector.tensor_copy(out=zs[:], in_=zred[:])

        # scalars: M=F[256], a=F[257], e=E[256], se=E[257] (all fp32)
        Mv = frow[0:1, FD:FD + 1]
        av = frow[0:1, FD + 1:FD + 2]
        ev = erow[0:1, FD:FD + 1]
        sev = erow[0:1, FD + 1:FD + 2]
        s_center = zs[0:1, 0:1]
        uv = zs[0:1, 1:2]
        psql = zs[0:1, 2:3]
        pbarlsq = zs[0:1, 3:4]
        fef = zs[0:1, 4:5]

        fin = keep.tile([1, 16], f32, name="fin")
        t_ = lambda i: fin[0:1, i:i + 1]
        # f0 = se*M ; f1 = a*e ; f2 = f0+f1
        nc.vector.tensor_tensor(out=t_(8), in0=sev, in1=Mv, op=Alu.mult)
        nc.vector.tensor_tensor(out=t_(9), in0=av, in1=ev, op=Alu.mult)
        nc.vector.tensor_tensor(out=t_(10), in0=t_(8), in1=t_(9), op=Alu.add)
        # f3 = -2*fef + f2
        nc.vector.tensor_scalar(out=t_(11), in0=fef, scalar1=-2.0,
                                scalar2=t_(10), op0=Alu.mult, op1=Alu.add)
        # f4 = f3 - psql ; f5 = f4 - pbarlsq
        nc.vector.tensor_tensor(out=t_(12), in0=t_(11), in1=psql, op=Alu.subtract)
        nc.vector.tensor_tensor(out=t_(13), in0=t_(12), in1=pbarlsq, op=Alu.subtract)
        # SD = 2*uv + f5
        nc.vector.tensor_scalar(out=t_(14), in0=uv, scalar1=2.0,
                                scalar2=t_(13), op0=Alu.mult, op1=Alu.add)
        # md = M*(M-1) ; rmd = 1/md ; div = SD*rmd*(-1/D)
        nc.vector.tensor_scalar(out=t_(15), in0=Mv, scalar1=-1.0,
                                scalar2=Mv, op0=Alu.add, op1=Alu.mult)
        nc.vector.reciprocal(t_(15), t_(15))
        nc.vector.tensor_tensor(out=t_(1), in0=t_(14), in1=t_(15), op=Alu.mult)
        nc.vector.tensor_scalar_mul(out=t_(1), in0=t_(1), scalar1=-1.0 / D)
        # tight = (a - s_center)/M
        nc.vector.tensor_tensor(out=t_(7), in0=av, in1=s_center, op=Alu.subtract)
        nc.vector.reciprocal(t_(6), Mv)
        nc.vector.tensor_tensor(out=t_(2), in0=t_(7), in1=t_(6), op=Alu.mult)
        # total = 0.1*div + 0.1*tight
        nc.vector.tensor_tensor(out=t_(0), in0=t_(1), in1=t_(2), op=Alu.add)
        nc.vector.tensor_scalar_mul(out=t_(0), in0=t_(0), scalar1=0.1)
        # debug slots
        nc.vector.tensor_copy(out=t_(3), in_=Mv)
        nc.vector.tensor_copy(out=t_(4), in_=av)
        nc.vector.tensor_copy(out=t_(5), in_=sev)

        nc.sync.dma_start(out=outd[None, :], in_=fin[0:1, 0:8])

    nc.finalize()
    return nc


def _get_compiled():
    if "nc" not in _compiled:
        _compiled["nc"] = _build_bass()
    return _compiled["nc"]


_IOTA = np.tile(np.arange(NSEG).astype(np.float16), (128, 1))


def _make_in_maps(features, targets, mask):
    features = np.ascontiguousarray(np.asarray(features, dtype=np.float32))
    targets = np.ascontiguousarray(np.asarray(targets, dtype=np.float32))
    maskf = np.asarray(mask).astype(np.float32).reshape(B, 1)
    in_maps = []
    for i in range(NCORES):
        sl = slice(i * RB, (i + 1) * RB)
        in_maps.append({
            "features": features[sl],
            "targets": targets[sl],
            "maskf": np.ascontiguousarray(maskf[sl]),
            "iotaperm": _IOTA,
        })
    return in_maps


def kernel(features, targets, mask):
    from concourse.bass_utils import run_bass_kernel_spmd

    nc = _get_compiled()
    in_maps = _make_in_maps(features, targets, mask)
    res = run_bass_kernel_spmd(nc, in_maps, list(range(NCORES)))
    out = res.results[0]["out"]
    total = np.float32(out[0])
    diversity = np.float32(out[1])
    tightness = np.float32(out[2])
    return total, diversity, tightness


# revision 32
# speedup vs baseline: 1.8328x; 1.0028x over previous
"""Trainium2 Bass kernel for CategoricalEntropyRegLoss.

Math: both loss terms factor so the [B,B] pairwise matrices are never built.

  feat_dists = sq_j + sq_k - 2 fn_j.fn_k            (rank FD+2)
  target_dists = (E_j - P_j.LQ_k) / D               (rank DC+1)
  S = sum_{jk} m_j m_k feat_dists * target_dists    (diag is exactly 0)
    = [ se*M + a*e - 2 Fe.F - Psq.L - Pbar.Lsq + 2 <U,V> ] / D
  tightness*M = a - sum_s ||seg_sum_s||^2 / max(cnt_s,1)

Everything needed is one matmul per core:
  out[1154, 258] = ext_seg^T @ ext_feat
  ext_seg  = [ onehot(code) | LQ | P | 1 | E ]      (B x 1154)
  ext_feat = [ m*fn | m | m*sq ]                    (B x 258)

Cross-core reduction: TWO pipelined NRT AllReduces. The stats rows
(130 x 258 fp32, 264KB) are ready ~10us before the segment one-hot
matmuls finish, so their AllReduce triggers early and absorbs the
one-time collective bring-up (~14-30us) while the segment matmuls run.
The segment AllReduce (1024 x 258 fp16, 528KB) queues right behind it;
a second collective starts ~1us after the first's mesh ends (measured).
The stats epilogue overlaps the second mesh.

(Alternatives measured and rejected: a remote-DMA SBUF exchange — each
blocking event-semaphore wait costs ~11-14us and a full-payload
allgather hits the ~45GB/s per-core DMA ceiling; a single combined
AllReduce — serializes the collective behind the last matmul and forces
one payload dtype.)

Front-end ordering: Ln table preloads during input DMA and the Lns run
first (stats path is the critical path); row sum-of-squares moves to
the otherwise-idle gpsimd; 1/norm uses one ACT Rsqrt; the argmax chain
is chunk-batched on DVE.

Precision: matmul operands fp16 (one-hot exact, 2x PE rate, PSUM fp32).
Segment rows travel fp16 (feeds only the squared-norm center term,
errors average over 1024 segments); stats travel fp32 (the diversity
total has ~7x cancellation; fp16 stats cost 1.6e-3 rel err, fp32 ~1e-5).
"""

import numpy as np

B = 4096
FD = 256
C = 32
D = 2
NSEG = C ** D          # 1024
NCORES = 8
RB = B // NCORES       # 512 rows per core
KT = RB // 128         # 4 k-chunks of 128 rows
EF = FD + 2            # 258: [mfn | m | m*sq]
ES = NSEG + 2 * D * C + 2   # 1154: [onehot | LQ | P | ones | E]
NMT = (ES + 127) // 128     # 10 m-tiles (last has 2 rows)

_compiled = {}


def _build_bass():
    from contextlib import ExitStack
    import concourse.bass as bass
    import concourse.bacc as bacc
    import concourse.tile as tile
    from concourse import mybir

    from concourse.tile import add_dep_helper

    f32 = mybir.dt.float32
    f16 = mybir.dt.float16
    Alu = mybir.AluOpType
    Act = mybir.ActivationFunctionType
    Ax = mybir.AxisListType

    nc = bacc.Bacc(num_devices=NCORES)

    feat = nc.dram_tensor("features", [RB, FD], f32, kind="ExternalInput")
    targ = nc.dram_tensor("targets", [RB, D * C], f32, kind="ExternalInput")
    maskf = nc.dram_tensor("maskf", [RB, 1], f32, kind="ExternalInput")
    # plain iota row, replicated to 128 partitions (cheaper than IOTA op)
    iotap = nc.dram_tensor("iotaperm", [128, NSEG], f16, kind="ExternalInput")
    outd = nc.dram_tensor("out", [8], f32, kind="ExternalOutput")

    with ExitStack() as ctx:
        tc = ctx.enter_context(tile.TileContext(nc))
        consts = ctx.enter_context(tc.tile_pool(name="consts", bufs=1))
        work = ctx.enter_context(tc.tile_pool(name="work", bufs=1))
        keep = ctx.enter_context(tc.tile_pool(name="keep", bufs=1))
        psum = ctx.enter_context(tc.tile_pool(name="psum", bufs=1, space="PSUM"))
        dram = ctx.enter_context(tc.tile_pool(name="dram", bufs=1, space="DRAM"))

        # ---------------- constants ----------------
        ones128 = consts.tile([128, 1], f32)
        nc.vector.memset(ones128[:], 1.0)

        # ---- batched input loads spread over the queues ----
        tbig = keep.tile([128, KT, D * C], f32, name="tbig")
        nc.scalar.dma_start(
            out=tbig[:], in_=targ[:, :].rearrange("(a p) f -> p a f", p=128))
        mkbig = keep.tile([128, KT, 1], f32, name="mkbig")
        nc.scalar.dma_start(
            out=mkbig[:], in_=maskf[:, :].rearrange("(a p) f -> p a f", p=128))
        # two tiles (not halves of one) so chunk reads only wait their own DMA
        xbig0 = keep.tile([128, 2, FD], f32, name="xbig0")
        nc.sync.dma_start(
            out=xbig0[:],
            in_=feat[0:256, :].rearrange("(a p) f -> p a f", p=128))
        xbig1 = keep.tile([128, 2, FD], f32, name="xbig1")
        nc.gpsimd.dma_start(
            out=xbig1[:],
            in_=feat[256:512, :].rearrange("(a p) f -> p a f", p=128))
        iota1024 = consts.tile([128, NSEG], f16)
        nc.sync.dma_start(out=iota1024[:], in_=iotap[:, :])

        def xchunk(kc):
            return xbig0[:, kc, :] if kc < 2 else xbig1[:, kc - 2, :]

        # biota[j] = 32 - j  (for first-argmax via reduce_max)
        biota = consts.tile([128, C], f32)
        nc.gpsimd.iota(biota[:], [[-1, C]], base=C, channel_multiplier=0,
                       allow_small_or_imprecise_dtypes=True)

        NST = 2 * D * C + 2   # 130 stats columns: [lq | p | ones | E]
        es_oh = [keep.tile([128, NSEG], f16, name=f"esoh_{kc}")
                 for kc in range(KT)]
        es_st = [keep.tile([128, NST], f16, name=f"esst_{kc}")
                 for kc in range(KT)]
        ef_16 = [keep.tile([128, EF], f16, name=f"eff_{kc}")
                 for kc in range(KT)]

        # ---- row sum-of-squares: squares on gpsimd (otherwise idle; keeps
        # ACT free for the Ln-first ordering), one batched DVE reduce
        # (gpsimd cannot reduce along the free axis) ----
        sqpack = keep.tile([128, KT], f32, name="sqpack")
        scrg4 = keep.tile([128, KT, FD], f32, name="scrg4")
        for kc in range(KT):
            nc.gpsimd.tensor_tensor(out=scrg4[:, kc, :], in0=xchunk(kc),
                                    in1=xchunk(kc), op=Alu.mult)
        nc.vector.reduce_sum(out=sqpack[:], in_=scrg4[:], axis=Ax.X)

        # ---- targets chains (DVE) ----
        # es_st columns: [0:64 lq | 64:128 p | 128 ones | 129 E]
        t1big = keep.tile([128, KT, D * C], f32, name="t1big")
        nc.vector.tensor_scalar_add(out=t1big[:], in0=tbig[:], scalar1=1e-10)
        invsb = keep.tile([128, KT * D], f32, name="invsb")
        nc.vector.reduce_sum(
            out=invsb[:],
            in_=t1big[:].rearrange("p a (d c) -> p (a d) c", c=C),
            axis=Ax.X)
        nc.vector.reciprocal(invsb[:], invsb[:])

        # ACT phase 1: Ln table preload (dummy) then the 4 Lns — the stats
        # m-tiles are the critical path (they feed the early AllReduce)
        lnscr = work.tile([128, 1], f32, name="lnscr", tag="lnscr")
        act_chain = [nc.scalar.activation(out=lnscr[:], in_=ones128[:],
                                          func=Act.Ln)]
        ln_acts = []
        for kc in range(KT):
            st_t = es_st[kc]
            pt = st_t[:, D * C:2 * D * C]
            for d_ in range(D):
                nc.vector.tensor_scalar_mul(
                    out=pt[:, C * d_:C * (d_ + 1)],
                    in0=t1big[:, kc, C * d_:C * (d_ + 1)],
                    scalar1=invsb[:, kc * D + d_:kc * D + d_ + 1])
            ln_acts.append(nc.scalar.activation(
                out=st_t[:, 0:D * C], in_=pt, func=Act.Ln))

        # E / ones columns right after each chunk's Ln
        for kc in range(KT):
            st_t = es_st[kc]
            scr64 = work.tile([128, D * C], f32, name=f"scr64_{kc}",
                              tag=f"s64_{kc}")
            nc.vector.tensor_tensor(out=scr64[:],
                                    in0=st_t[:, D * C:2 * D * C],
                                    in1=st_t[:, 0:D * C], op=Alu.mult)
            escr = work.tile([128, 1], f32, name=f"escr_{kc}",
                             tag=f"es_{kc}")
            nc.vector.reduce_sum(out=escr[:], in_=scr64[:], axis=Ax.X)
            nc.vector.tensor_copy(out=st_t[:, NST - 1:NST], in_=escr[:])
            nc.vector.memset(st_t[:, NST - 2:NST - 1], 1.0)

        # ---- 1/norm: one ACT Sqrt + DVE reciprocal (phase 2) ----
        nc.vector.tensor_scalar_max(out=sqpack[:], in0=sqpack[:],
                                    scalar1=1e-24)
        normpack = keep.tile([128, KT], f32, name="normpack")
        act_chain.append(nc.scalar.sqrt(normpack[:], sqpack[:]))
        invpack = keep.tile([128, KT], f32, name="invpack")
        nc.vector.reciprocal(invpack[:], normpack[:])
        minvpack = keep.tile([128, KT], f32, name="minvpack")
        nc.vector.tensor_tensor(out=minvpack[:], in0=invpack[:],
                                in1=mkbig[:, :, 0], op=Alu.mult)

        # ---- ext_feat = [x*(m*inv) | m | sq*inv*minv] (ACT phase 3) ----
        copy_acts = []
        for kc in range(KT):
            ef_t = ef_16[kc]
            copy_acts.append(nc.scalar.activation(
                out=ef_t[:, 0:FD], in_=xchunk(kc), func=Act.Copy,
                scale=minvpack[:, kc:kc + 1]))
            nc.vector.tensor_copy(out=ef_t[:, FD:FD + 1], in_=mkbig[:, kc, :])
            nc.vector.tensor_scalar(out=ef_t[:, FD + 1:FD + 2],
                                    in0=sqpack[:, kc:kc + 1],
                                    scalar1=invpack[:, kc:kc + 1],
                                    scalar2=minvpack[:, kc:kc + 1],
                                    op0=Alu.mult, op1=Alu.mult)

        # ---- chunk-batched first-argmax, then code = cls0 + 32*cls1 ----
        AD = KT * D   # 8 (kc, d) groups
        mx8 = work.tile([128, AD], f32, name="mx8", tag="mx8")
        nc.vector.reduce_max(
            out=mx8[:],
            in_=t1big[:].rearrange("p a (d c) -> p (a d) c", c=C),
            axis=Ax.X)
        cand8 = work.tile([128, AD, C], f32, name="cand8", tag="cand8")
        for kc in range(KT):
            for d_ in range(D):
                g = kc * D + d_
                # (t1 == max) * (32 - idx); reduce_max -> 32 - first_argmax
                nc.vector.scalar_tensor_tensor(
                    out=cand8[:, g, :],
                    in0=t1big[:, kc, C * d_:C * (d_ + 1)],
                    scalar=mx8[:, g:g + 1], in1=biota[:],
                    op0=Alu.is_equal, op1=Alu.mult)
        mq8 = work.tile([128, AD], f32, name="mq8", tag="mq8")
        nc.vector.reduce_max(out=mq8[:], in_=cand8[:], axis=Ax.X)
        cls8 = work.tile([128, AD], f32, name="cls8", tag="cls8")
        nc.vector.tensor_scalar(out=cls8[:], in0=mq8[:], scalar1=-1.0,
                                scalar2=float(C), op0=Alu.mult, op1=Alu.add)
        # code4[kc] = cls[kc,0] + 32*cls[kc,1]
        code4 = work.tile([128, KT], f32, name="code4", tag="code4")
        cls_v = cls8[:].rearrange("p (a two) -> p a two", two=2)
        nc.vector.tensor_scalar_mul(out=code4[:], in0=cls_v[:, :, 1],
                                    scalar1=float(C))
        nc.vector.tensor_tensor(out=code4[:], in0=code4[:],
                                in1=cls_v[:, :, 0], op=Alu.add)
        for kc in range(KT):
            nc.vector.tensor_scalar(
                out=es_oh[kc][:], in0=iota1024[:],
                scalar1=code4[:, kc:kc + 1],
                scalar2=None, op0=Alu.is_equal)

        # keep ACT ops grouped by function (avoid act-table reload thrash)
        act_chain = (act_chain[:1] + ln_acts + act_chain[1:] + copy_acts)
        for a, b in zip(act_chain[1:], act_chain[:-1]):
            add_dep_helper(a.ins, b.ins, sync=False,
                           reason="act table grouping")

        # ---------------- payload tiles + AllReduce buffers ----------------
        # stats packed [64, 3, 258]: slot0 = LQ rows, slot1 = P rows (pair i
        # on partition i), slot2 = F row (p0) + E row (p1), rest zero.
        seg_pay = keep.tile([128, 8, EF], f16, name="seg_pay")
        st_pay = keep.tile([64, 3, EF], f32, name="st_pay")
        nc.vector.memset(st_pay[:, 2:3, :], 0.0)
        inb_st = dram.tile([64, 3, EF], f32, name="inb_st")
        outb_st = dram.tile([64, 3, EF], f32, name="outb_st",
                            addr_space="Shared")
        inb_seg = dram.tile([128, 8, EF], f16, name="inb_seg")
        outb_seg = dram.tile([128, 8, EF], f16, name="outb_seg",
                             addr_space="Shared")

        # ---------------- matmuls ----------------------
        # Both payloads are ready long before the wall-clock floor (~55us)
        # at which the first mesh can begin, so order the ARs for epilogue
        # overlap: segments FIRST (their heavy epilogue hides inside the
        # stats mesh), packed stats second.
        for mt in range(8):
            mlo = mt * 128
            ps = psum.tile([128, EF], f32, name=f"ps_{mt}", tag=f"ps_{mt % 7}")
            for kc in range(KT):
                nc.tensor.matmul(out=ps[:], lhsT=es_oh[kc][:, mlo:mlo + 128],
                                 rhs=ef_16[kc][:],
                                 start=(kc == 0), stop=(kc == KT - 1))
            # alternate engines so copies keep pace with the matmuls
            # (gpsimd cannot read PSUM; scalar's Copy is table-less)
            if mt % 2 == 0:
                nc.vector.tensor_copy(out=seg_pay[:, mt, :], in_=ps[:])
            else:
                nc.scalar.activation(out=seg_pay[:, mt, :], in_=ps[:],
                                     func=Act.Copy)
        nc.sync.dma_start(out=inb_seg[:], in_=seg_pay[:])
        nc.gpsimd.collective_compute(
            "AllReduce", mybir.AluOpType.add,
            replica_groups=[list(range(NCORES))],
            ins=[inb_seg.opt()], outs=[outb_seg.opt()])

        # stats m-tiles: LQ and P as separate m=64 chunks so the pair rows
        # land partition-aligned in slots 0/1 (no re-basing DMA later)
        psA = psum.tile([64, EF], f32, name="psA", tag="ps_0")
        psB = psum.tile([64, EF], f32, name="psB", tag="ps_1")
        psC = psum.tile([2, EF], f32, name="psC", tag="ps_2")
        for kc in range(KT):
            st = (kc == 0)
            sp = (kc == KT - 1)
            nc.tensor.matmul(out=psA[:], lhsT=es_st[kc][:, 0:64],
                             rhs=ef_16[kc][:], start=st, stop=sp)
            nc.tensor.matmul(out=psB[:], lhsT=es_st[kc][:, 64:128],
                             rhs=ef_16[kc][:], start=st, stop=sp)
            nc.tensor.matmul(out=psC[:], lhsT=es_st[kc][:, 128:130],
                             rhs=ef_16[kc][:], start=st, stop=sp)
        nc.vector.tensor_copy(out=st_pay[:, 0, :], in_=psA[:])
        nc.vector.tensor_copy(out=st_pay[:, 1, :], in_=psB[:])
        nc.vector.tensor_copy(out=st_pay[0:2, 2, :], in_=psC[0:2, :])
        nc.sync.dma_start(out=inb_st[:], in_=st_pay[:])
        nc.gpsimd.collective_compute(
            "AllReduce", mybir.AluOpType.add,
            replica_groups=[list(range(NCORES))],
            ins=[inb_st.opt()], outs=[outb_st.opt()])

        # ---------------- segment epilogue (hides in the stats mesh) ------
        # loads split over two queues; squares split ACT/DVE
        big0 = keep.tile([128, 4, EF], f16, name="big0")
        nc.sync.dma_start(out=big0[:], in_=outb_seg[:, 0:4, :])
        big1 = keep.tile([128, 4, EF], f16, name="big1")
        nc.scalar.dma_start(out=big1[:], in_=outb_seg[:, 4:8, :])

        Z = keep.tile([128, 8], f32, name="Z")
        nc.vector.memset(Z[:], 0.0)

        nrmp = keep.tile([128, 8], f32, name="nrmp")
        sq_acts = []
        for s in range(4):
            sq_acts.append(nc.scalar.activation(
                out=scrg4[:, 0, :], in_=big1[:, s, 0:FD], func=Act.Square,
                accum_out=nrmp[:, 4 + s:5 + s]))
        for a, b in zip(sq_acts[1:], sq_acts[:-1]):
            add_dep_helper(a.ins, b.ins, sync=False, reason="act grouping")
        scrB = keep.tile([128, 4, FD], f32, name="scrB")
        nc.vector.tensor_tensor(out=scrB[:], in0=big0[:, :, 0:FD],
                                in1=big0[:, :, 0:FD], op=Alu.mult)
        nc.vector.reduce_sum(out=nrmp[:, 0:4], in_=scrB[:], axis=Ax.X)
        cdp = keep.tile([128, 8], f32, name="cdp")
        nc.vector.tensor_scalar_max(out=cdp[:, 0:4], in0=big0[:, :, FD],
                                    scalar1=1.0)
        nc.vector.tensor_scalar_max(out=cdp[:, 4:8], in0=big1[:, :, FD],
                                    scalar1=1.0)
        rcdp = keep.tile([128, 8], f32, name="rcdp")
        nc.vector.reciprocal(rcdp[:], cdp[:])
        termp = keep.tile([128, 8], f32, name="termp")
        nc.vector.tensor_tensor(out=termp[:], in0=nrmp[:], in1=rcdp[:],
                                op=Alu.mult)
        nc.vector.reduce_sum(out=Z[:, 0:1], in_=termp[:], axis=Ax.X)

        # ---------------- stats epilogue (after the second mesh) ----------
        ut = keep.tile([64, EF], f32, name="ut")
        nc.sync.dma_start(out=ut[:], in_=outb_st[0:64, 0, :])
        vt = keep.tile([64, EF], f32, name="vt")
        nc.sync.dma_start(out=vt[:], in_=outb_st[0:64, 1, :])
        frow = keep.tile([1, EF], f32, name="frow")
        nc.scalar.dma_start(out=frow[:], in_=outb_st[0:1, 2, :])
        erow = keep.tile([1, EF], f32, name="erow")
        nc.scalar.dma_start(out=erow[:], in_=outb_st[1:2, 2, :])

        scrU = keep.tile([64, FD], f32, name="scrU")
        nc.vector.tensor_tensor(out=scrU[:], in0=ut[:, 0:FD],
                                in1=vt[:, 0:FD], op=Alu.mult)
        nc.vector.reduce_sum(out=Z[0:64, 1:2], in_=scrU[:], axis=Ax.X)
        nc.vector.tensor_tensor(out=Z[0:64, 2:3], in0=vt[:, FD + 1:FD + 2],
                                in1=ut[:, FD:FD + 1], op=Alu.mult)     # Psq*L
        nc.vector.tensor_tensor(out=Z[0:64, 3:4], in0=vt[:, FD:FD + 1],
                                in1=ut[:, FD + 1:FD + 2], op=Alu.mult)  # Pbar*Lsq
        scrF = keep.tile([1, FD], f32, name="scrF")
        nc.vector.tensor_tensor(out=scrF[:], in0=frow[0:1, 0:FD],
                                in1=erow[0:1, 0:FD], op=Alu.mult)
        nc.vector.reduce_sum(out=Z[0:1, 4:5], in_=scrF[:], axis=Ax.X)  # Fe.F

        zred = psum.tile([1, 8], f32, name="zred", tag="ps_3")
        nc.tensor.matmul(out=zred[:], lhsT=ones128[:], rhs=Z[:],
                         start=True, stop=True)
        zs = keep.tile([1, 8], f32, name="zs")
        nc.vector.tensor_copy(out=zs[:], in_=zred[:])

        # scalars: M=F[256], a=F[257], e=E[256], se=E[257] (all fp32)
        Mv = frow[0:1, FD:FD + 1]
        av = frow[0:1, FD + 1:FD + 2]
        ev = erow[0:1, FD:FD + 1]
        sev = erow[0:1, FD + 1:FD + 2]
        s_center = zs[0:1, 0:1]
        uv = zs[0:1, 1:2]
        psql = zs[0:1, 2:3]
        pbarlsq = zs[0:1, 3:4]
        fef = zs[0:1, 4:5]

        fin = keep.tile([1, 16], f32, name="fin")
        t_ = lambda i: fin[0:1, i:i + 1]
        # f0 = se*M ; f1 = a*e ; f2 = f0+f1
        nc.vector.tensor_tensor(out=t_(8), in0=sev, in1=Mv, op=Alu.mult)
        nc.vector.tensor_tensor(out=t_(9), in0=av, in1=ev, op=Alu.mult)
        nc.vector.tensor_tensor(out=t_(10), in0=t_(8), in1=t_(9), op=Alu.add)
        # f3 = -2*fef + f2
        nc.vector.tensor_scalar(out=t_(11), in0=fef, scalar1=-2.0,
                                scalar2=t_(10), op0=Alu.mult, op1=Alu.add)
        # f4 = f3 - psql ; f5 = f4 - pbarlsq
        nc.vector.tensor_tensor(out=t_(12), in0=t_(11), in1=psql, op=Alu.subtract)
        nc.vector.tensor_tensor(out=t_(13), in0=t_(12), in1=pbarlsq, op=Alu.subtract)
        # SD = 2*uv + f5
        nc.vector.tensor_scalar(out=t_(14), in0=uv, scalar1=2.0,
                                scalar2=t_(13), op0=Alu.mult, op1=Alu.add)
        # md = M*(M-1) ; rmd = 1/md ; div = SD*rmd*(-1/D)
        nc.vector.tensor_scalar(out=t_(15), in0=Mv, scalar1=-1.0,
                                scalar2=Mv, op0=Alu.add, op1=Alu.mult)
        nc.vector.reciprocal(t_(15), t_(15))
        nc.vector.tensor_tensor(out=t_(1), in0=t_(14), in1=t_(15), op=Alu.mult)
        nc.vector.tensor_scalar_mul(out=t_(1), in0=t_(1), scalar1=-1.0 / D)
        # tight = (a - s_center)/M
        nc.vector.tensor_tensor(out=t_(7), in0=av, in1=s_center, op=Alu.subtract)
        nc.vector.reciprocal(t_(6), Mv)
        nc.vector.tensor_tensor(out=t_(2), in0=t_(7), in1=t_(6), op=Alu.mult)
        # total = 0.1*div + 0.1*tight
        nc.vector.tensor_tensor(out=t_(0), in0=t_(1), in1=t_(2), op=Alu.add)
        nc.vector.tensor_scalar_mul(out=t_(0), in0=t_(0), scalar1=0.1)
        # debug slots
        nc.vector.tensor_copy(out=t_(3), in_=Mv)
        nc.vector.tensor_copy(out=t_(4), in_=av)
        nc.vector.tensor_copy(out=t_(5), in_=sev)

        nc.sync.dma_start(out=outd[None, :], in_=fin[0:1, 0:8])

    nc.finalize()
    return nc


def _get_compiled():
    if "nc" not in _compiled:
        _compiled["nc"] = _build_bass()
    return _compiled["nc"]


_IOTA = np.tile(np.arange(NSEG).astype(np.float16), (128, 1))


def _make_in_maps(features, targets, mask):
    features = np.ascontiguousarray(np.asarray(features, dtype=np.float32))
    targets = np.ascontiguousarray(np.asarray(targets, dtype=np.float32))
    maskf = np.asarray(mask).astype(np.float32).reshape(B, 1)
    in_maps = []
    for i in range(NCORES):
        sl = slice(i * RB, (i + 1) * RB)
        in_maps.append({
            "features": features[sl],
            "targets": targets[sl],
            "maskf": np.ascontiguousarray(maskf[sl]),
            "iotaperm": _IOTA,
        })
    return in_maps


def kernel(features, targets, mask):
    from concourse.bass_utils import run_bass_kernel_spmd

    nc = _get_compiled()
    in_maps = _make_in_maps(features, targets, mask)
    res = run_bass_kernel_spmd(nc, in_maps, list(range(NCORES)))
    out = res.results[0]["out"]
    total = np.float32(out[0])
    diversity = np.float32(out[1])
    tightness = np.float32(out[2])
    return total, diversity, tightness


# revision 33
# speedup vs baseline: 1.8641x; 1.0170x over previous
"""Trainium2 Bass kernel for CategoricalEntropyRegLoss.

Math: both loss terms factor so the [B,B] pairwise matrices are never built.

  feat_dists = sq_j + sq_k - 2 fn_j.fn_k            (rank FD+2)
  target_dists = (E_j - P_j.LQ_k) / D               (rank DC+1)
  S = sum_{jk} m_j m_k feat_dists * target_dists    (diag is exactly 0)
    = [ se*M + a*e - 2 Fe.F - Psq.L - Pbar.Lsq + 2 <U,V> ] / D
  tightness*M = a - sum_s ||seg_sum_s||^2 / max(cnt_s,1)

Everything needed is one matmul per core:
  out[1154, 258] = ext_seg^T @ ext_feat
  ext_seg  = [ onehot(code) | LQ | P | 1 | E ]      (B x 1154)
  ext_feat = [ m*fn | m | m*sq ]                    (B x 258)

Cross-core reduction: TWO pipelined NRT AllReduces. The stats rows
(130 x 258 fp32, 264KB) are ready ~10us before the segment one-hot
matmuls finish, so their AllReduce triggers early and absorbs the
one-time collective bring-up (~14-30us) while the segment matmuls run.
The segment AllReduce (1024 x 258 fp16, 528KB) queues right behind it;
a second collective starts ~1us after the first's mesh ends (measured).
The stats epilogue overlaps the second mesh.

(Alternatives measured and rejected: a remote-DMA SBUF exchange — each
blocking event-semaphore wait costs ~11-14us and a full-payload
allgather hits the ~45GB/s per-core DMA ceiling; a single combined
AllReduce — serializes the collective behind the last matmul and forces
one payload dtype.)

Front-end ordering: Ln table preloads during input DMA and the Lns run
first (stats path is the critical path); row sum-of-squares moves to
the otherwise-idle gpsimd; 1/norm uses one ACT Rsqrt; the argmax chain
is chunk-batched on DVE.

Precision: matmul operands fp16 (one-hot exact, 2x PE rate, PSUM fp32).
Segment rows travel fp16 (feeds only the squared-norm center term,
errors average over 1024 segments); stats travel fp32 (the diversity
total has ~7x cancellation; fp16 stats cost 1.6e-3 rel err, fp32 ~1e-5).
"""

import numpy as np

B = 4096
FD = 256
C = 32
D = 2
NSEG = C ** D          # 1024
NCORES = 8
RB = B // NCORES       # 512 rows per core
KT = RB // 128         # 4 k-chunks of 128 rows
EF = FD + 2            # 258: [mfn | m | m*sq]
ES = NSEG + 2 * D * C + 2   # 1154: [onehot | LQ | P | ones | E]
NMT = (ES + 127) // 128     # 10 m-tiles (last has 2 rows)

_compiled = {}


def _build_bass():
    from contextlib import ExitStack
    import concourse.bass as bass
    import concourse.bacc as bacc
    import concourse.tile as tile
    from concourse import mybir

    from concourse.tile import add_dep_helper

    f32 = mybir.dt.float32
    f16 = mybir.dt.float16
    Alu = mybir.AluOpType
    Act = mybir.ActivationFunctionType
    Ax = mybir.AxisListType

    nc = bacc.Bacc(num_devices=NCORES)

    feat = nc.dram_tensor("features", [RB, FD], f16, kind="ExternalInput")
    targ = nc.dram_tensor("targets", [RB, D * C], f32, kind="ExternalInput")
    maskf = nc.dram_tensor("maskf", [RB, 1], f32, kind="ExternalInput")
    outd = nc.dram_tensor("out", [8], f32, kind="ExternalOutput")

    with ExitStack() as ctx:
        tc = ctx.enter_context(tile.TileContext(nc))
        consts = ctx.enter_context(tc.tile_pool(name="consts", bufs=1))
        work = ctx.enter_context(tc.tile_pool(name="work", bufs=1))
        keep = ctx.enter_context(tc.tile_pool(name="keep", bufs=1))
        psum = ctx.enter_context(tc.tile_pool(name="psum", bufs=1, space="PSUM"))
        dram = ctx.enter_context(tc.tile_pool(name="dram", bufs=1, space="DRAM"))

        # ---------------- constants ----------------
        ones128 = consts.tile([128, 1], f32)
        nc.vector.memset(ones128[:], 1.0)

        # ---- batched input loads spread over the queues ----
        tbig = keep.tile([128, KT, D * C], f32, name="tbig")
        nc.scalar.dma_start(
            out=tbig[:], in_=targ[:, :].rearrange("(a p) f -> p a f", p=128))
        mkbig = keep.tile([128, KT, 1], f32, name="mkbig")
        nc.scalar.dma_start(
            out=mkbig[:], in_=maskf[:, :].rearrange("(a p) f -> p a f", p=128))
        # two tiles (not halves of one) so chunk reads only wait their own DMA
        xbig0 = keep.tile([128, 2, FD], f16, name="xbig0")
        nc.sync.dma_start(
            out=xbig0[:],
            in_=feat[0:256, :].rearrange("(a p) f -> p a f", p=128))
        xbig1 = keep.tile([128, 2, FD], f16, name="xbig1")
        nc.gpsimd.dma_start(
            out=xbig1[:],
            in_=feat[256:512, :].rearrange("(a p) f -> p a f", p=128))

        def xchunk(kc):
            return xbig0[:, kc, :] if kc < 2 else xbig1[:, kc - 2, :]

        iota1024 = consts.tile([128, NSEG], f32)
        nc.gpsimd.iota(iota1024[:], [[1, NSEG]], channel_multiplier=0,
                       allow_small_or_imprecise_dtypes=True)
        # biota[j] = 32 - j  (for first-argmax via reduce_max)
        biota = consts.tile([128, C], f32)
        nc.gpsimd.iota(biota[:], [[-1, C]], base=C, channel_multiplier=0,
                       allow_small_or_imprecise_dtypes=True)

        NST = 2 * D * C + 2   # 130 stats columns: [lq | p | ones | E]
        es_oh = [keep.tile([128, NSEG], f16, name=f"esoh_{kc}")
                 for kc in range(KT)]
        es_st = [keep.tile([128, NST], f16, name=f"esst_{kc}")
                 for kc in range(KT)]
        ef_16 = [keep.tile([128, EF], f16, name=f"eff_{kc}")
                 for kc in range(KT)]

        # ---- row sum-of-squares: squares on gpsimd (otherwise idle; keeps
        # ACT free for the Ln-first ordering), one batched DVE reduce
        # (gpsimd cannot reduce along the free axis) ----
        sqpack = keep.tile([128, KT], f32, name="sqpack")
        scrg4 = keep.tile([128, KT, FD], f32, name="scrg4")
        for kc in range(KT):
            nc.gpsimd.tensor_tensor(out=scrg4[:, kc, :], in0=xchunk(kc),
                                    in1=xchunk(kc), op=Alu.mult)
        nc.vector.reduce_sum(out=sqpack[:], in_=scrg4[:], axis=Ax.X)

        # ---- targets chains (DVE) ----
        # es_st columns: [0:64 lq | 64:128 p | 128 ones | 129 E]
        t1big = keep.tile([128, KT, D * C], f32, name="t1big")
        nc.vector.tensor_scalar_add(out=t1big[:], in0=tbig[:], scalar1=1e-10)
        invsb = keep.tile([128, KT * D], f32, name="invsb")
        nc.vector.reduce_sum(
            out=invsb[:],
            in_=t1big[:].rearrange("p a (d c) -> p (a d) c", c=C),
            axis=Ax.X)
        nc.vector.reciprocal(invsb[:], invsb[:])

        # ACT phase 1: Ln table preload (dummy) then the 4 Lns — the stats
        # m-tiles are the critical path (they feed the early AllReduce)
        lnscr = work.tile([128, 1], f32, name="lnscr", tag="lnscr")
        act_chain = [nc.scalar.activation(out=lnscr[:], in_=ones128[:],
                                          func=Act.Ln)]
        ln_acts = []
        for kc in range(KT):
            st_t = es_st[kc]
            pt = st_t[:, D * C:2 * D * C]
            for d_ in range(D):
                nc.vector.tensor_scalar_mul(
                    out=pt[:, C * d_:C * (d_ + 1)],
                    in0=t1big[:, kc, C * d_:C * (d_ + 1)],
                    scalar1=invsb[:, kc * D + d_:kc * D + d_ + 1])
            ln_acts.append(nc.scalar.activation(
                out=st_t[:, 0:D * C], in_=pt, func=Act.Ln))

        # E / ones columns right after each chunk's Ln
        for kc in range(KT):
            st_t = es_st[kc]
            scr64 = work.tile([128, D * C], f32, name=f"scr64_{kc}",
                              tag=f"s64_{kc}")
            nc.vector.tensor_tensor(out=scr64[:],
                                    in0=st_t[:, D * C:2 * D * C],
                                    in1=st_t[:, 0:D * C], op=Alu.mult)
            escr = work.tile([128, 1], f32, name=f"escr_{kc}",
                             tag=f"es_{kc}")
            nc.vector.reduce_sum(out=escr[:], in_=scr64[:], axis=Ax.X)
            nc.vector.tensor_copy(out=st_t[:, NST - 1:NST], in_=escr[:])
            nc.vector.memset(st_t[:, NST - 2:NST - 1], 1.0)

        # ---- 1/norm: one ACT Sqrt + DVE reciprocal (phase 2) ----
        nc.vector.tensor_scalar_max(out=sqpack[:], in0=sqpack[:],
                                    scalar1=1e-24)
        normpack = keep.tile([128, KT], f32, name="normpack")
        act_chain.append(nc.scalar.sqrt(normpack[:], sqpack[:]))
        invpack = keep.tile([128, KT], f32, name="invpack")
        nc.vector.reciprocal(invpack[:], normpack[:])
        minvpack = keep.tile([128, KT], f32, name="minvpack")
        nc.vector.tensor_tensor(out=minvpack[:], in0=invpack[:],
                                in1=mkbig[:, :, 0], op=Alu.mult)

        # ---- ext_feat = [x*(m*inv) | m | sq*inv*minv] (ACT phase 3) ----
        copy_acts = []
        for kc in range(KT):
            ef_t = ef_16[kc]
            copy_acts.append(nc.scalar.activation(
                out=ef_t[:, 0:FD], in_=xchunk(kc), func=Act.Copy,
                scale=minvpack[:, kc:kc + 1]))
            nc.vector.tensor_copy(out=ef_t[:, FD:FD + 1], in_=mkbig[:, kc, :])
            nc.vector.tensor_scalar(out=ef_t[:, FD + 1:FD + 2],
                                    in0=sqpack[:, kc:kc + 1],
                                    scalar1=invpack[:, kc:kc + 1],
                                    scalar2=minvpack[:, kc:kc + 1],
                                    op0=Alu.mult, op1=Alu.mult)

        # ---- chunk-batched first-argmax, then code = cls0 + 32*cls1 ----
        AD = KT * D   # 8 (kc, d) groups
        mx8 = work.tile([128, AD], f32, name="mx8", tag="mx8")
        nc.vector.reduce_max(
            out=mx8[:],
            in_=t1big[:].rearrange("p a (d c) -> p (a d) c", c=C),
            axis=Ax.X)
        cand8 = work.tile([128, AD, C], f32, name="cand8", tag="cand8")
        for kc in range(KT):
            for d_ in range(D):
                g = kc * D + d_
                # (t1 == max) * (32 - idx); reduce_max -> 32 - first_argmax
                nc.vector.scalar_tensor_tensor(
                    out=cand8[:, g, :],
                    in0=t1big[:, kc, C * d_:C * (d_ + 1)],
                    scalar=mx8[:, g:g + 1], in1=biota[:],
                    op0=Alu.is_equal, op1=Alu.mult)
        mq8 = work.tile([128, AD], f32, name="mq8", tag="mq8")
        nc.vector.reduce_max(out=mq8[:], in_=cand8[:], axis=Ax.X)
        cls8 = work.tile([128, AD], f32, name="cls8", tag="cls8")
        nc.vector.tensor_scalar(out=cls8[:], in0=mq8[:], scalar1=-1.0,
                                scalar2=float(C), op0=Alu.mult, op1=Alu.add)
        # code4[kc] = cls[kc,0] + 32*cls[kc,1]
        code4 = work.tile([128, KT], f32, name="code4", tag="code4")
        cls_v = cls8[:].rearrange("p (a two) -> p a two", two=2)
        nc.vector.tensor_scalar_mul(out=code4[:], in0=cls_v[:, :, 1],
                                    scalar1=float(C))
        nc.vector.tensor_tensor(out=code4[:], in0=code4[:],
                                in1=cls_v[:, :, 0], op=Alu.add)
        for kc in range(KT):
            nc.vector.tensor_scalar(
                out=es_oh[kc][:], in0=iota1024[:],
                scalar1=code4[:, kc:kc + 1],
                scalar2=None, op0=Alu.is_equal)

        # keep ACT ops grouped by function (avoid act-table reload thrash)
        act_chain = (act_chain[:1] + ln_acts + act_chain[1:] + copy_acts)
        for a, b in zip(act_chain[1:], act_chain[:-1]):
            add_dep_helper(a.ins, b.ins, sync=False,
                           reason="act table grouping")

        # ---------------- payload tiles + AllReduce buffers ----------------
        # stats packed [64, 3, 258]: slot0 = LQ rows, slot1 = P rows (pair i
        # on partition i), slot2 = F row (p0) + E row (p1), rest zero.
        seg_pay = keep.tile([128, 8, EF], f16, name="seg_pay")
        st_pay = keep.tile([64, 3, EF], f32, name="st_pay")
        nc.vector.memset(st_pay[:, 2:3, :], 0.0)
        inb_st = dram.tile([64, 3, EF], f32, name="inb_st")
        outb_st = dram.tile([64, 3, EF], f32, name="outb_st",
                            addr_space="Shared")
        inb_seg = dram.tile([128, 8, EF], f16, name="inb_seg")
        outb_seg = dram.tile([128, 8, EF], f16, name="outb_seg",
                             addr_space="Shared")

        # ---------------- matmuls ----------------------
        # Both payloads are ready long before the wall-clock floor (~55us)
        # at which the first mesh can begin, so order the ARs for epilogue
        # overlap: segments FIRST (their heavy epilogue hides inside the
        # stats mesh), packed stats second.
        for mt in range(8):
            mlo = mt * 128
            ps = psum.tile([128, EF], f32, name=f"ps_{mt}", tag=f"ps_{mt % 7}")
            for kc in range(KT):
                nc.tensor.matmul(out=ps[:], lhsT=es_oh[kc][:, mlo:mlo + 128],
                                 rhs=ef_16[kc][:],
                                 start=(kc == 0), stop=(kc == KT - 1))
            # alternate engines so copies keep pace with the matmuls
            # (gpsimd cannot read PSUM; scalar's Copy is table-less)
            if mt % 2 == 0:
                nc.vector.tensor_copy(out=seg_pay[:, mt, :], in_=ps[:])
            else:
                nc.scalar.activation(out=seg_pay[:, mt, :], in_=ps[:],
                                     func=Act.Copy)
        nc.sync.dma_start(out=inb_seg[:], in_=seg_pay[:])
        nc.gpsimd.collective_compute(
            "AllReduce", mybir.AluOpType.add,
            replica_groups=[list(range(NCORES))],
            ins=[inb_seg.opt()], outs=[outb_seg.opt()])

        # stats m-tiles: LQ and P as separate m=64 chunks so the pair rows
        # land partition-aligned in slots 0/1 (no re-basing DMA later)
        psA = psum.tile([64, EF], f32, name="psA", tag="ps_0")
        psB = psum.tile([64, EF], f32, name="psB", tag="ps_1")
        psC = psum.tile([2, EF], f32, name="psC", tag="ps_2")
        for kc in range(KT):
            st = (kc == 0)
            sp = (kc == KT - 1)
            nc.tensor.matmul(out=psA[:], lhsT=es_st[kc][:, 0:64],
                             rhs=ef_16[kc][:], start=st, stop=sp)
            nc.tensor.matmul(out=psB[:], lhsT=es_st[kc][:, 64:128],
                             rhs=ef_16[kc][:], start=st, stop=sp)
            nc.tensor.matmul(out=psC[:], lhsT=es_st[kc][:, 128:130],
                             rhs=ef_16[kc][:], start=st, stop=sp)
        nc.vector.tensor_copy(out=st_pay[:, 0, :], in_=psA[:])
        nc.vector.tensor_copy(out=st_pay[:, 1, :], in_=psB[:])
        nc.vector.tensor_copy(out=st_pay[0:2, 2, :], in_=psC[0:2, :])
        nc.sync.dma_start(out=inb_st[:], in_=st_pay[:])
        nc.gpsimd.collective_compute(
            "AllReduce", mybir.AluOpType.add,
            replica_groups=[list(range(NCORES))],
            ins=[inb_st.opt()], outs=[outb_st.opt()])

        # ---------------- segment epilogue (hides in the stats mesh) ------
        # loads split over two queues; squares split ACT/DVE
        big0 = keep.tile([128, 4, EF], f16, name="big0")
        nc.sync.dma_start(out=big0[:], in_=outb_seg[:, 0:4, :])
        big1 = keep.tile([128, 4, EF], f16, name="big1")
        nc.scalar.dma_start(out=big1[:], in_=outb_seg[:, 4:8, :])

        Z = keep.tile([128, 8], f32, name="Z")
        nc.vector.memset(Z[:], 0.0)

        nrmp = keep.tile([128, 8], f32, name="nrmp")
        sq_acts = []
        for s in range(4):
            sq_acts.append(nc.scalar.activation(
                out=scrg4[:, 0, :], in_=big1[:, s, 0:FD], func=Act.Square,
                accum_out=nrmp[:, 4 + s:5 + s]))
        for a, b in zip(sq_acts[1:], sq_acts[:-1]):
            add_dep_helper(a.ins, b.ins, sync=False, reason="act grouping")
        scrB = keep.tile([128, 4, FD], f32, name="scrB")
        nc.vector.tensor_tensor(out=scrB[:], in0=big0[:, :, 0:FD],
                                in1=big0[:, :, 0:FD], op=Alu.mult)
        nc.vector.reduce_sum(out=nrmp[:, 0:4], in_=scrB[:], axis=Ax.X)
        cdp = keep.tile([128, 8], f32, name="cdp")
        nc.vector.tensor_scalar_max(out=cdp[:, 0:4], in0=big0[:, :, FD],
                                    scalar1=1.0)
        nc.vector.tensor_scalar_max(out=cdp[:, 4:8], in0=big1[:, :, FD],
                                    scalar1=1.0)
        rcdp = keep.tile([128, 8], f32, name="rcdp")
        nc.vector.reciprocal(rcdp[:], cdp[:])
        termp = keep.tile([128, 8], f32, name="termp")
        nc.vector.tensor_tensor(out=termp[:], in0=nrmp[:], in1=rcdp[:],
                                op=Alu.mult)
        nc.vector.reduce_sum(out=Z[:, 0:1], in_=termp[:], axis=Ax.X)

        # ---------------- stats epilogue (after the second mesh) ----------
        stall = keep.tile([64, 3, EF], f32, name="stall")
        nc.sync.dma_start(out=stall[:], in_=outb_st[:, :, :])
        erow = keep.tile([1, EF], f32, name="erow")
        nc.scalar.dma_start(out=erow[:], in_=outb_st[1:2, 2, :])
        ut = stall[:, 0, :]
        vt = stall[:, 1, :]
        frow = stall[0:1, 2, :]

        scrU = keep.tile([64, FD], f32, name="scrU")
        nc.vector.tensor_tensor(out=scrU[:], in0=ut[:, 0:FD],
                                in1=vt[:, 0:FD], op=Alu.mult)
        nc.vector.reduce_sum(out=Z[0:64, 1:2], in_=scrU[:], axis=Ax.X)
        nc.vector.tensor_tensor(out=Z[0:64, 2:3], in0=vt[:, FD + 1:FD + 2],
                                in1=ut[:, FD:FD + 1], op=Alu.mult)     # Psq*L
        nc.vector.tensor_tensor(out=Z[0:64, 3:4], in0=vt[:, FD:FD + 1],
                                in1=ut[:, FD + 1:FD + 2], op=Alu.mult)  # Pbar*Lsq
        scrF = keep.tile([1, FD], f32, name="scrF")
        nc.vector.tensor_tensor(out=scrF[:], in0=frow[0:1, 0:FD],
                                in1=erow[0:1, 0:FD], op=Alu.mult)
        nc.vector.reduce_sum(out=Z[0:1, 4:5], in_=scrF[:], axis=Ax.X)  # Fe.F

        zred = psum.tile([1, 8], f32, name="zred", tag="ps_3")
        nc.tensor.matmul(out=zred[:], lhsT=ones128[:], rhs=Z[:],
                         start=True, stop=True)
        zs = keep.tile([1, 8], f32, name="zs")
        nc.vector.tensor_copy(out=zs[:], in_=zred[:])

        # scalars: M=F[256], a=F[257], e=E[256], se=E[257] (all fp32)
        Mv = frow[0:1, FD:FD + 1]
        av = frow[0:1, FD + 1:FD + 2]
        ev = erow[0:1, FD:FD + 1]
        sev = erow[0:1, FD + 1:FD + 2]
        s_center = zs[0:1, 0:1]
        uv = zs[0:1, 1:2]
        psql = zs[0:1, 2:3]
        pbarlsq = zs[0:1, 3:4]
        fef = zs[0:1, 4:5]

        fin = keep.tile([1, 16], f32, name="fin")
        t_ = lambda i: fin[0:1, i:i + 1]
        # f0 = se*M ; f1 = a*e ; f2 = f0+f1
        nc.vector.tensor_tensor(out=t_(8), in0=sev, in1=Mv, op=Alu.mult)
        nc.vector.tensor_tensor(out=t_(9), in0=av, in1=ev, op=Alu.mult)
        nc.vector.tensor_tensor(out=t_(10), in0=t_(8), in1=t_(9), op=Alu.add)
        # f3 = -2*fef + f2
        nc.vector.tensor_scalar(out=t_(11), in0=fef, scalar1=-2.0,
                                scalar2=t_(10), op0=Alu.mult, op1=Alu.add)
        # f4 = f3 - psql ; f5 = f4 - pbarlsq
        nc.vector.tensor_tensor(out=t_(12), in0=t_(11), in1=psql, op=Alu.subtract)
        nc.vector.tensor_tensor(out=t_(13), in0=t_(12), in1=pbarlsq, op=Alu.subtract)
        # SD = 2*uv + f5
        nc.vector.tensor_scalar(out=t_(14), in0=uv, scalar1=2.0,
                                scalar2=t_(13), op0=Alu.mult, op1=Alu.add)
        # md = M*(M-1) ; rmd = 1/md ; div = SD*rmd*(-1/D)
        nc.vector.tensor_scalar(out=t_(15), in0=Mv, scalar1=-1.0,
                                scalar2=Mv, op0=Alu.add, op1=Alu.mult)
        nc.vector.reciprocal(t_(15), t_(15))
        nc.vector.tensor_tensor(out=t_(1), in0=t_(14), in1=t_(15), op=Alu.mult)
        nc.vector.tensor_scalar_mul(out=t_(1), in0=t_(1), scalar1=-1.0 / D)
        # tight = (a - s_center)/M
        nc.vector.tensor_tensor(out=t_(7), in0=av, in1=s_center, op=Alu.subtract)
        nc.vector.reciprocal(t_(6), Mv)
        nc.vector.tensor_tensor(out=t_(2), in0=t_(7), in1=t_(6), op=Alu.mult)
        # total = 0.1*div + 0.1*tight
        nc.vector.tensor_tensor(out=t_(0), in0=t_(1), in1=t_(2), op=Alu.add)
        nc.vector.tensor_scalar_mul(out=t_(0), in0=t_(0), scalar1=0.1)
        # debug slots
        nc.vector.tensor_copy(out=t_(3), in_=Mv)
        nc.vector.tensor_copy(out=t_(4), in_=av)
        nc.vector.tensor_copy(out=t_(5), in_=sev)

        nc.sync.dma_start(out=outd[None, :], in_=fin[0:1, 0:8])

    nc.finalize()
    return nc


def _get_compiled():
    if "nc" not in _compiled:
        _compiled["nc"] = _build_bass()
    return _compiled["nc"]


def _make_in_maps(features, targets, mask):
    features = np.ascontiguousarray(np.asarray(features).astype(np.float16))
    targets = np.ascontiguousarray(np.asarray(targets, dtype=np.float32))
    maskf = np.asarray(mask).astype(np.float32).reshape(B, 1)
    in_maps = []
    for i in range(NCORES):
        sl = slice(i * RB, (i + 1) * RB)
        in_maps.append({
            "features": features[sl],
            "targets": targets[sl],
            "maskf": np.ascontiguousarray(maskf[sl]),
        })
    return in_maps


def kernel(features, targets, mask):
    from concourse.bass_utils import run_bass_kernel_spmd

    nc = _get_compiled()
    in_maps = _make_in_maps(features, targets, mask)
    res = run_bass_kernel_spmd(nc, in_maps, list(range(NCORES)))
    out = res.results[0]["out"]
    total = np.float32(out[0])
    diversity = np.float32(out[1])
    tightness = np.float32(out[2])
    return total, diversity, tightness


# revision 34
# speedup vs baseline: 1.9166x; 1.0282x over previous
"""Trainium2 Bass kernel for CategoricalEntropyRegLoss.

Math: both loss terms factor so the [B,B] pairwise matrices are never built.

  feat_dists = sq_j + sq_k - 2 fn_j.fn_k            (rank FD+2)
  target_dists = (E_j - P_j.LQ_k) / D               (rank DC+1)
  S = sum_{jk} m_j m_k feat_dists * target_dists    (diag is exactly 0)
    = [ se*M + a*e - 2 Fe.F - Psq.L - Pbar.Lsq + 2 <U,V> ] / D
  tightness*M = a - sum_s ||seg_sum_s||^2 / max(cnt_s,1)

Everything needed is one matmul per core:
  out[1154, 258] = ext_seg^T @ ext_feat
  ext_seg  = [ onehot(code) | LQ | P | 1 | E ]      (B x 1154)
  ext_feat = [ m*fn | m | m*sq ]                    (B x 258)

Cross-core reduction: TWO pipelined NRT AllReduces. The stats rows
(130 x 258 fp32, 264KB) are ready ~10us before the segment one-hot
matmuls finish, so their AllReduce triggers early and absorbs the
one-time collective bring-up (~14-30us) while the segment matmuls run.
The segment AllReduce (1024 x 258 fp16, 528KB) queues right behind it;
a second collective starts ~1us after the first's mesh ends (measured).
The stats epilogue overlaps the second mesh.

(Alternatives measured and rejected: a remote-DMA SBUF exchange — each
blocking event-semaphore wait costs ~11-14us and a full-payload
allgather hits the ~45GB/s per-core DMA ceiling; a single combined
AllReduce — serializes the collective behind the last matmul and forces
one payload dtype.)

Front-end ordering: Ln table preloads during input DMA and the Lns run
first (stats path is the critical path); row sum-of-squares moves to
the otherwise-idle gpsimd; 1/norm uses one ACT Rsqrt; the argmax chain
is chunk-batched on DVE.

Precision: matmul operands fp16 (one-hot exact, 2x PE rate, PSUM fp32).
Segment rows travel fp16 (feeds only the squared-norm center term,
errors average over 1024 segments); stats travel fp32 (the diversity
total has ~7x cancellation; fp16 stats cost 1.6e-3 rel err, fp32 ~1e-5).
"""

import numpy as np

B = 4096
FD = 256
C = 32
D = 2
NSEG = C ** D          # 1024
NCORES = 8
RB = B // NCORES       # 512 rows per core
KT = RB // 128         # 4 k-chunks of 128 rows
EF = FD + 2            # 258: [mfn | m | m*sq]
ES = NSEG + 2 * D * C + 2   # 1154: [onehot | LQ | P | ones | E]
NMT = (ES + 127) // 128     # 10 m-tiles (last has 2 rows)

_compiled = {}


def _build_bass():
    from contextlib import ExitStack
    import concourse.bass as bass
    import concourse.bacc as bacc
    import concourse.tile as tile
    from concourse import mybir

    from concourse.tile import add_dep_helper

    f32 = mybir.dt.float32
    f16 = mybir.dt.float16
    Alu = mybir.AluOpType
    Act = mybir.ActivationFunctionType
    Ax = mybir.AxisListType

    nc = bacc.Bacc(num_devices=NCORES)

    feat = nc.dram_tensor("features", [RB, FD], f16, kind="ExternalInput")
    targ = nc.dram_tensor("targets", [RB, D * C], f32, kind="ExternalInput")
    maskf = nc.dram_tensor("maskf", [RB, 1], f32, kind="ExternalInput")
    outd = nc.dram_tensor("out", [8], f32, kind="ExternalOutput")

    with ExitStack() as ctx:
        tc = ctx.enter_context(tile.TileContext(nc))
        consts = ctx.enter_context(tc.tile_pool(name="consts", bufs=1))
        work = ctx.enter_context(tc.tile_pool(name="work", bufs=1))
        keep = ctx.enter_context(tc.tile_pool(name="keep", bufs=1))
        psum = ctx.enter_context(tc.tile_pool(name="psum", bufs=1, space="PSUM"))
        dram = ctx.enter_context(tc.tile_pool(name="dram", bufs=1, space="DRAM"))

        # ---------------- constants ----------------
        ones128 = consts.tile([128, 1], f32)
        nc.vector.memset(ones128[:], 1.0)

        # ---- batched input loads spread over the queues ----
        tbig = keep.tile([128, KT, D * C], f32, name="tbig")
        nc.scalar.dma_start(
            out=tbig[:], in_=targ[:, :].rearrange("(a p) f -> p a f", p=128))
        mkbig = keep.tile([128, KT, 1], f32, name="mkbig")
        nc.scalar.dma_start(
            out=mkbig[:], in_=maskf[:, :].rearrange("(a p) f -> p a f", p=128))
        # two tiles (not halves of one) so chunk reads only wait their own DMA
        xbig0 = keep.tile([128, 2, FD], f16, name="xbig0")
        nc.sync.dma_start(
            out=xbig0[:],
            in_=feat[0:256, :].rearrange("(a p) f -> p a f", p=128))
        xbig1 = keep.tile([128, 2, FD], f16, name="xbig1")
        nc.gpsimd.dma_start(
            out=xbig1[:],
            in_=feat[256:512, :].rearrange("(a p) f -> p a f", p=128))

        def xchunk(kc):
            return xbig0[:, kc, :] if kc < 2 else xbig1[:, kc - 2, :]

        iota1024 = consts.tile([128, NSEG], f32)
        nc.gpsimd.iota(iota1024[:], [[1, NSEG]], channel_multiplier=0,
                       allow_small_or_imprecise_dtypes=True)
        # biota[j] = 32 - j  (for first-argmax via reduce_max)
        biota = consts.tile([128, C], f32)
        nc.gpsimd.iota(biota[:], [[-1, C]], base=C, channel_multiplier=0,
                       allow_small_or_imprecise_dtypes=True)

        NST = 2 * D * C + 2   # 130 stats columns: [lq | p | ones | E]
        es_oh = [keep.tile([128, NSEG], f16, name=f"esoh_{kc}")
                 for kc in range(KT)]
        es_st = [keep.tile([128, NST], f16, name=f"esst_{kc}")
                 for kc in range(KT)]
        ef_16 = [keep.tile([128, EF], f16, name=f"eff_{kc}")
                 for kc in range(KT)]

        # ---- row sum-of-squares: squares on gpsimd (otherwise idle; keeps
        # ACT free for the Ln-first ordering), one batched DVE reduce
        # (gpsimd cannot reduce along the free axis) ----
        sqpack = keep.tile([128, KT], f32, name="sqpack")
        scrg4 = keep.tile([128, KT, FD], f32, name="scrg4")
        for kc in range(KT):
            nc.gpsimd.tensor_tensor(out=scrg4[:, kc, :], in0=xchunk(kc),
                                    in1=xchunk(kc), op=Alu.mult)
        nc.vector.reduce_sum(out=sqpack[:], in_=scrg4[:], axis=Ax.X)

        # ---- targets chains (DVE) ----
        # es_st columns: [0:64 lq | 64:128 p | 128 ones | 129 E]
        t1big = keep.tile([128, KT, D * C], f32, name="t1big")
        nc.vector.tensor_scalar_add(out=t1big[:], in0=tbig[:], scalar1=1e-10)
        invsb = keep.tile([128, KT * D], f32, name="invsb")
        nc.vector.reduce_sum(
            out=invsb[:],
            in_=t1big[:].rearrange("p a (d c) -> p (a d) c", c=C),
            axis=Ax.X)
        nc.vector.reciprocal(invsb[:], invsb[:])

        # ACT phase 1: Ln table preload (dummy) then the 4 Lns — the stats
        # m-tiles are the critical path (they feed the early AllReduce)
        lnscr = work.tile([128, 1], f32, name="lnscr", tag="lnscr")
        act_chain = [nc.scalar.activation(out=lnscr[:], in_=ones128[:],
                                          func=Act.Ln)]
        ln_acts = []
        for kc in range(KT):
            st_t = es_st[kc]
            pt = st_t[:, D * C:2 * D * C]
            for d_ in range(D):
                nc.vector.tensor_scalar_mul(
                    out=pt[:, C * d_:C * (d_ + 1)],
                    in0=t1big[:, kc, C * d_:C * (d_ + 1)],
                    scalar1=invsb[:, kc * D + d_:kc * D + d_ + 1])
            ln_acts.append(nc.scalar.activation(
                out=st_t[:, 0:D * C], in_=pt, func=Act.Ln))

        # E / ones columns right after each chunk's Ln
        for kc in range(KT):
            st_t = es_st[kc]
            scr64 = work.tile([128, D * C], f32, name=f"scr64_{kc}",
                              tag=f"s64_{kc}")
            nc.vector.tensor_tensor(out=scr64[:],
                                    in0=st_t[:, D * C:2 * D * C],
                                    in1=st_t[:, 0:D * C], op=Alu.mult)
            escr = work.tile([128, 1], f32, name=f"escr_{kc}",
                             tag=f"es_{kc}")
            nc.vector.reduce_sum(out=escr[:], in_=scr64[:], axis=Ax.X)
            nc.vector.tensor_copy(out=st_t[:, NST - 1:NST], in_=escr[:])
            nc.vector.memset(st_t[:, NST - 2:NST - 1], 1.0)

        # ---- 1/norm: one ACT Sqrt + DVE reciprocal (phase 2) ----
        nc.vector.tensor_scalar_max(out=sqpack[:], in0=sqpack[:],
                                    scalar1=1e-24)
        normpack = keep.tile([128, KT], f32, name="normpack")
        act_chain.append(nc.scalar.sqrt(normpack[:], sqpack[:]))
        invpack = keep.tile([128, KT], f32, name="invpack")
        nc.vector.reciprocal(invpack[:], normpack[:])
        minvpack = keep.tile([128, KT], f32, name="minvpack")
        nc.vector.tensor_tensor(out=minvpack[:], in0=invpack[:],
                                in1=mkbig[:, :, 0], op=Alu.mult)

        # ---- ext_feat = [x*(m*inv) | m | sq*inv*minv] (ACT phase 3) ----
        copy_acts = []
        for kc in range(KT):
            ef_t = ef_16[kc]
            copy_acts.append(nc.scalar.activation(
                out=ef_t[:, 0:FD], in_=xchunk(kc), func=Act.Copy,
                scale=minvpack[:, kc:kc + 1]))
            nc.vector.tensor_copy(out=ef_t[:, FD:FD + 1], in_=mkbig[:, kc, :])
            nc.vector.tensor_scalar(out=ef_t[:, FD + 1:FD + 2],
                                    in0=sqpack[:, kc:kc + 1],
                                    scalar1=invpack[:, kc:kc + 1],
                                    scalar2=minvpack[:, kc:kc + 1],
                                    op0=Alu.mult, op1=Alu.mult)

        # ---- chunk-batched first-argmax, then code = cls0 + 32*cls1 ----
        AD = KT * D   # 8 (kc, d) groups
        mx8 = work.tile([128, AD], f32, name="mx8", tag="mx8")
        nc.vector.reduce_max(
            out=mx8[:],
            in_=t1big[:].rearrange("p a (d c) -> p (a d) c", c=C),
            axis=Ax.X)
        cand8 = work.tile([128, AD, C], f32, name="cand8", tag="cand8")
        for kc in range(KT):
            for d_ in range(D):
                g = kc * D + d_
                # (t1 == max) * (32 - idx); reduce_max -> 32 - first_argmax
                nc.vector.scalar_tensor_tensor(
                    out=cand8[:, g, :],
                    in0=t1big[:, kc, C * d_:C * (d_ + 1)],
                    scalar=mx8[:, g:g + 1], in1=biota[:],
                    op0=Alu.is_equal, op1=Alu.mult)
        mq8 = work.tile([128, AD], f32, name="mq8", tag="mq8")
        nc.vector.reduce_max(out=mq8[:], in_=cand8[:], axis=Ax.X)
        cls8 = work.tile([128, AD], f32, name="cls8", tag="cls8")
        nc.vector.tensor_scalar(out=cls8[:], in0=mq8[:], scalar1=-1.0,
                                scalar2=float(C), op0=Alu.mult, op1=Alu.add)
        # code4[kc] = cls[kc,0] + 32*cls[kc,1]
        code4 = work.tile([128, KT], f32, name="code4", tag="code4")
        cls_v = cls8[:].rearrange("p (a two) -> p a two", two=2)
        nc.vector.tensor_scalar_mul(out=code4[:], in0=cls_v[:, :, 1],
                                    scalar1=float(C))
        nc.vector.tensor_tensor(out=code4[:], in0=code4[:],
                                in1=cls_v[:, :, 0], op=Alu.add)
        for kc in range(KT):
            nc.vector.tensor_scalar(
                out=es_oh[kc][:], in0=iota1024[:],
                scalar1=code4[:, kc:kc + 1],
                scalar2=None, op0=Alu.is_equal)

        # keep ACT ops grouped by function (avoid act-table reload thrash)
        act_chain = (act_chain[:1] + ln_acts + act_chain[1:] + copy_acts)
        for a, b in zip(act_chain[1:], act_chain[:-1]):
            add_dep_helper(a.ins, b.ins, sync=False,
                           reason="act table grouping")

        # ---------------- payload tiles + AllReduce buffers ----------------
        # stats packed [64, 3, 258]: slot0 = LQ rows, slot1 = P rows (pair i
        # on partition i), slot2 = F row (p0) + E row (p1), rest zero.
        seg_pay = keep.tile([128, 8, EF], f16, name="seg_pay")
        st_pay = keep.tile([64, 3, EF], f32, name="st_pay")
        nc.vector.memset(st_pay[:, 2:3, :], 0.0)
        inb_st = dram.tile([64, 3, EF], f32, name="inb_st")
        outb_st = dram.tile([64, 3, EF], f32, name="outb_st",
                            addr_space="Shared")
        inb_seg = dram.tile([128, 8, EF], f16, name="inb_seg")
        outb_seg = dram.tile([128, 8, EF], f16, name="outb_seg",
                             addr_space="Shared")

        # ---------------- matmuls ----------------------
        # Both payloads are ready long before the wall-clock floor (~55us)
        # at which the first mesh can begin, so order the ARs for epilogue
        # overlap: segments FIRST (their heavy epilogue hides inside the
        # stats mesh), packed stats second.
        for mt in range(8):
            mlo = mt * 128
            ps = psum.tile([128, EF], f32, name=f"ps_{mt}", tag=f"ps_{mt % 7}")
            for kc in range(KT):
                nc.tensor.matmul(out=ps[:], lhsT=es_oh[kc][:, mlo:mlo + 128],
                                 rhs=ef_16[kc][:],
                                 start=(kc == 0), stop=(kc == KT - 1))
            # alternate engines so copies keep pace with the matmuls
            # (gpsimd cannot read PSUM; scalar's Copy is table-less)
            if mt % 2 == 0:
                nc.vector.tensor_copy(out=seg_pay[:, mt, :], in_=ps[:])
            else:
                nc.scalar.activation(out=seg_pay[:, mt, :], in_=ps[:],
                                     func=Act.Copy)
        nc.sync.dma_start(out=inb_seg[:], in_=seg_pay[:])
        nc.gpsimd.collective_compute(
            "AllReduce", mybir.AluOpType.add,
            replica_groups=[list(range(NCORES))],
            ins=[inb_seg.opt()], outs=[outb_seg.opt()])

        # stats m-tiles: LQ and P as separate m=64 chunks so the pair rows
        # land partition-aligned in slots 0/1 (no re-basing DMA later)
        psA = psum.tile([64, EF], f32, name="psA", tag="ps_0")
        psB = psum.tile([64, EF], f32, name="psB", tag="ps_1")
        psC = psum.tile([2, EF], f32, name="psC", tag="ps_2")
        for kc in range(KT):
            st = (kc == 0)
            sp = (kc == KT - 1)
            nc.tensor.matmul(out=psA[:], lhsT=es_st[kc][:, 0:64],
                             rhs=ef_16[kc][:], start=st, stop=sp)
            nc.tensor.matmul(out=psB[:], lhsT=es_st[kc][:, 64:128],
                             rhs=ef_16[kc][:], start=st, stop=sp)
            nc.tensor.matmul(out=psC[:], lhsT=es_st[kc][:, 128:130],
                             rhs=ef_16[kc][:], start=st, stop=sp)
        nc.vector.tensor_copy(out=st_pay[:, 0, :], in_=psA[:])
        nc.vector.tensor_copy(out=st_pay[:, 1, :], in_=psB[:])
        nc.vector.tensor_copy(out=st_pay[0:2, 2, :], in_=psC[0:2, :])
        nc.sync.dma_start(out=inb_st[:], in_=st_pay[:])
        nc.gpsimd.collective_compute(
            "AllReduce", mybir.AluOpType.add,
            replica_groups=[list(range(NCORES))],
            ins=[inb_st.opt()], outs=[outb_st.opt()])

        # ---------------- segment epilogue (hides in the stats mesh) ------
        # loads split over two queues; squares split ACT/DVE
        big0 = keep.tile([128, 4, EF], f16, name="big0")
        nc.sync.dma_start(out=big0[:], in_=outb_seg[:, 0:4, :])
        big1 = keep.tile([128, 4, EF], f16, name="big1")
        nc.scalar.dma_start(out=big1[:], in_=outb_seg[:, 4:8, :])

        Z = keep.tile([128, 8], f32, name="Z")
        nc.vector.memset(Z[:], 0.0)

        nrmp = keep.tile([128, 8], f32, name="nrmp")
        sq_acts = []
        for s in range(4):
            sq_acts.append(nc.scalar.activation(
                out=scrg4[:, 0, :], in_=big1[:, s, 0:FD], func=Act.Square,
                accum_out=nrmp[:, 4 + s:5 + s]))
        for a, b in zip(sq_acts[1:], sq_acts[:-1]):
            add_dep_helper(a.ins, b.ins, sync=False, reason="act grouping")
        scrB = keep.tile([128, 4, FD], f32, name="scrB")
        nc.vector.tensor_tensor(out=scrB[:], in0=big0[:, :, 0:FD],
                                in1=big0[:, :, 0:FD], op=Alu.mult)
        nc.vector.reduce_sum(out=nrmp[:, 0:4], in_=scrB[:], axis=Ax.X)
        cdp = keep.tile([128, 8], f32, name="cdp")
        nc.vector.tensor_scalar_max(out=cdp[:, 0:4], in0=big0[:, :, FD],
                                    scalar1=1.0)
        nc.vector.tensor_scalar_max(out=cdp[:, 4:8], in0=big1[:, :, FD],
                                    scalar1=1.0)
        rcdp = keep.tile([128, 8], f32, name="rcdp")
        nc.vector.reciprocal(rcdp[:], cdp[:])
        termp = keep.tile([128, 8], f32, name="termp")
        nc.vector.tensor_tensor(out=termp[:], in0=nrmp[:], in1=rcdp[:],
                                op=Alu.mult)
        nc.vector.reduce_sum(out=Z[:, 0:1], in_=termp[:], axis=Ax.X)

        # ---------------- stats epilogue (after the second mesh) ----------
        stall = keep.tile([64, 3, EF], f32, name="stall")
        nc.sync.dma_start(out=stall[:], in_=outb_st[:, :, :])
        erow = keep.tile([1, EF], f32, name="erow")
        nc.sync.dma_start(out=erow[:], in_=outb_st[1:2, 2, :])
        # F and E feature rows transposed to 128 partitions x 2 so Fe.F is a
        # lane-parallel multiply that rides the ones-matmul (columns 4:6)
        frT = keep.tile([128, 2], f32, name="frT")
        nc.scalar.dma_start(
            out=frT[:],
            in_=outb_st[0:1, 2, 0:FD].rearrange("o (a p) -> p (o a)", p=128))
        erT = keep.tile([128, 2], f32, name="erT")
        nc.scalar.dma_start(
            out=erT[:],
            in_=outb_st[1:2, 2, 0:FD].rearrange("o (a p) -> p (o a)", p=128))
        ut = stall[:, 0, :]
        vt = stall[:, 1, :]
        frow = stall[0:1, 2, :]

        scrU = keep.tile([64, FD], f32, name="scrU")
        nc.vector.tensor_tensor(out=scrU[:], in0=ut[:, 0:FD],
                                in1=vt[:, 0:FD], op=Alu.mult)
        nc.vector.reduce_sum(out=Z[0:64, 1:2], in_=scrU[:], axis=Ax.X)
        nc.vector.tensor_tensor(out=Z[0:64, 2:3], in0=vt[:, FD + 1:FD + 2],
                                in1=ut[:, FD:FD + 1], op=Alu.mult)     # Psq*L
        nc.vector.tensor_tensor(out=Z[0:64, 3:4], in0=vt[:, FD:FD + 1],
                                in1=ut[:, FD + 1:FD + 2], op=Alu.mult)  # Pbar*Lsq
        nc.vector.tensor_tensor(out=Z[:, 4:6], in0=frT[:],
                                in1=erT[:], op=Alu.mult)               # Fe.F

        zred = psum.tile([1, 8], f32, name="zred", tag="ps_3")
        nc.tensor.matmul(out=zred[:], lhsT=ones128[:], rhs=Z[:],
                         start=True, stop=True)
        zs = keep.tile([1, 8], f32, name="zs")
        nc.vector.tensor_copy(out=zs[:], in_=zred[:])

        # scalars: M=F[256], a=F[257], e=E[256], se=E[257] (all fp32)
        Mv = frow[0:1, FD:FD + 1]
        av = frow[0:1, FD + 1:FD + 2]
        ev = erow[0:1, FD:FD + 1]
        sev = erow[0:1, FD + 1:FD + 2]
        s_center = zs[0:1, 0:1]
        uv = zs[0:1, 1:2]
        psql = zs[0:1, 2:3]
        pbarlsq = zs[0:1, 3:4]
        fef = zs[0:1, 6:7]
        nc.vector.tensor_tensor(out=fef, in0=zs[0:1, 4:5],
                                in1=zs[0:1, 5:6], op=Alu.add)

        fin = keep.tile([1, 16], f32, name="fin")
        t_ = lambda i: fin[0:1, i:i + 1]
        # f0 = se*M ; f1 = a*e ; f2 = f0+f1
        nc.vector.tensor_tensor(out=t_(8), in0=sev, in1=Mv, op=Alu.mult)
        nc.vector.tensor_tensor(out=t_(9), in0=av, in1=ev, op=Alu.mult)
        nc.vector.tensor_tensor(out=t_(10), in0=t_(8), in1=t_(9), op=Alu.add)
        # f3 = -2*fef + f2
        nc.vector.tensor_scalar(out=t_(11), in0=fef, scalar1=-2.0,
                                scalar2=t_(10), op0=Alu.mult, op1=Alu.add)
        # f4 = f3 - psql ; f5 = f4 - pbarlsq
        nc.vector.tensor_tensor(out=t_(12), in0=t_(11), in1=psql, op=Alu.subtract)
        nc.vector.tensor_tensor(out=t_(13), in0=t_(12), in1=pbarlsq, op=Alu.subtract)
        # SD = 2*uv + f5
        nc.vector.tensor_scalar(out=t_(14), in0=uv, scalar1=2.0,
                                scalar2=t_(13), op0=Alu.mult, op1=Alu.add)
        # md = M*(M-1) ; rmd = 1/md ; div = SD*rmd*(-1/D)
        nc.vector.tensor_scalar(out=t_(15), in0=Mv, scalar1=-1.0,
                                scalar2=Mv, op0=Alu.add, op1=Alu.mult)
        nc.vector.reciprocal(t_(15), t_(15))
        nc.vector.tensor_tensor(out=t_(1), in0=t_(14), in1=t_(15), op=Alu.mult)
        nc.vector.tensor_scalar_mul(out=t_(1), in0=t_(1), scalar1=-1.0 / D)
        # tight = (a - s_center)/M
        nc.vector.tensor_tensor(out=t_(7), in0=av, in1=s_center, op=Alu.subtract)
        nc.vector.reciprocal(t_(6), Mv)
        nc.vector.tensor_tensor(out=t_(2), in0=t_(7), in1=t_(6), op=Alu.mult)
        # total = 0.1*div + 0.1*tight
        nc.vector.tensor_tensor(out=t_(0), in0=t_(1), in1=t_(2), op=Alu.add)
        nc.vector.tensor_scalar_mul(out=t_(0), in0=t_(0), scalar1=0.1)
        nc.sync.dma_start(out=outd[None, :], in_=fin[0:1, 0:8])

    nc.finalize()
    return nc


def _get_compiled():
    if "nc" not in _compiled:
        _compiled["nc"] = _build_bass()
    return _compiled["nc"]


def _make_in_maps(features, targets, mask):
    features = np.ascontiguousarray(np.asarray(features).astype(np.float16))
    targets = np.ascontiguousarray(np.asarray(targets, dtype=np.float32))
    maskf = np.asarray(mask).astype(np.float32).reshape(B, 1)
    in_maps = []
    for i in range(NCORES):
        sl = slice(i * RB, (i + 1) * RB)
        in_maps.append({
            "features": features[sl],
            "targets": targets[sl],
            "maskf": np.ascontiguousarray(maskf[sl]),
        })
    return in_maps


def kernel(features, targets, mask):
    from concourse.bass_utils import run_bass_kernel_spmd

    nc = _get_compiled()
    in_maps = _make_in_maps(features, targets, mask)
    res = run_bass_kernel_spmd(nc, in_maps, list(range(NCORES)))
    out = res.results[0]["out"]
    total = np.float32(out[0])
    diversity = np.float32(out[1])
    tightness = np.float32(out[2])
    return total, diversity, tightness


# revision 35
# speedup vs baseline: 2.2467x; 1.1722x over previous
"""Trainium2 Bass kernel for CategoricalEntropyRegLoss.

Math: both loss terms factor so the [B,B] pairwise matrices are never built.

  feat_dists = sq_j + sq_k - 2 fn_j.fn_k            (rank FD+2)
  target_dists = (E_j - P_j.LQ_k) / D               (rank DC+1)
  S = sum_{jk} m_j m_k feat_dists * target_dists    (diag is exactly 0)
    = [ se*M + a*e - 2 Fe.F - Psq.L - Pbar.Lsq + 2 <U,V> ] / D
  tightness*M = a - sum_s ||seg_sum_s||^2 / max(cnt_s,1)

Everything needed is one matmul per core:
  out[1154, 258] = ext_seg^T @ ext_feat
  ext_seg  = [ onehot(code) | LQ | P | 1 | E ]      (B x 1154)
  ext_feat = [ m*fn | m | m*sq ]                    (B x 258)

Cross-core reduction: TWO pipelined NRT AllReduces. The stats rows
(130 x 258 fp32, 264KB) are ready ~10us before the segment one-hot
matmuls finish, so their AllReduce triggers early and absorbs the
one-time collective bring-up (~14-30us) while the segment matmuls run.
The segment AllReduce (1024 x 258 fp16, 528KB) queues right behind it;
a second collective starts ~1us after the first's mesh ends (measured).
The stats epilogue overlaps the second mesh.

(Alternatives measured and rejected: a remote-DMA SBUF exchange — each
blocking event-semaphore wait costs ~11-14us and a full-payload
allgather hits the ~45GB/s per-core DMA ceiling; a single combined
AllReduce — serializes the collective behind the last matmul and forces
one payload dtype.)

Front-end ordering: Ln table preloads during input DMA and the Lns run
first (stats path is the critical path); row sum-of-squares moves to
the otherwise-idle gpsimd; 1/norm uses one ACT Rsqrt; the argmax chain
is chunk-batched on DVE.

Precision: matmul operands fp16 (one-hot exact, 2x PE rate, PSUM fp32).
Segment rows travel fp16 (feeds only the squared-norm center term,
errors average over 1024 segments); stats travel fp32 (the diversity
total has ~7x cancellation; fp16 stats cost 1.6e-3 rel err, fp32 ~1e-5).
"""

import numpy as np

B = 4096
FD = 256
C = 32
D = 2
NSEG = C ** D          # 1024
NCORES = 8
RB = B // NCORES       # 512 rows per core
KT = RB // 128         # 4 k-chunks of 128 rows
EF = FD + 2            # 258: [mfn | m | m*sq]
ES = NSEG + 2 * D * C + 2   # 1154: [onehot | LQ | P | ones | E]
NMT = (ES + 127) // 128     # 10 m-tiles (last has 2 rows)

_compiled = {}


def _build_bass():
    from contextlib import ExitStack
    import concourse.bass as bass
    import concourse.bacc as bacc
    import concourse.tile as tile
    from concourse import mybir

    from concourse.tile import add_dep_helper

    f32 = mybir.dt.float32
    f16 = mybir.dt.float16
    Alu = mybir.AluOpType
    Act = mybir.ActivationFunctionType
    Ax = mybir.AxisListType

    nc = bacc.Bacc(num_devices=NCORES)

    feat = nc.dram_tensor("features", [RB, FD], f16, kind="ExternalInput")
    targ = nc.dram_tensor("targets", [RB, D * C], f32, kind="ExternalInput")
    maskf = nc.dram_tensor("maskf", [RB, 1], f32, kind="ExternalInput")
    outd = nc.dram_tensor("out", [8], f32, kind="ExternalOutput")

    with ExitStack() as ctx:
        tc = ctx.enter_context(tile.TileContext(nc))
        consts = ctx.enter_context(tc.tile_pool(name="consts", bufs=1))
        work = ctx.enter_context(tc.tile_pool(name="work", bufs=1))
        keep = ctx.enter_context(tc.tile_pool(name="keep", bufs=1))
        psum = ctx.enter_context(tc.tile_pool(name="psum", bufs=1, space="PSUM"))
        dram = ctx.enter_context(tc.tile_pool(name="dram", bufs=1, space="DRAM"))

        # ---------------- constants ----------------
        ones128 = consts.tile([128, 1], f32)
        nc.vector.memset(ones128[:], 1.0)

        # ---- batched input loads spread over the queues ----
        tbig = keep.tile([128, KT, D * C], f32, name="tbig")
        nc.scalar.dma_start(
            out=tbig[:], in_=targ[:, :].rearrange("(a p) f -> p a f", p=128))
        mkbig = keep.tile([128, KT, 1], f32, name="mkbig")
        nc.scalar.dma_start(
            out=mkbig[:], in_=maskf[:, :].rearrange("(a p) f -> p a f", p=128))
        # two tiles (not halves of one) so chunk reads only wait their own DMA
        xbig0 = keep.tile([128, 2, FD], f16, name="xbig0")
        nc.sync.dma_start(
            out=xbig0[:],
            in_=feat[0:256, :].rearrange("(a p) f -> p a f", p=128))
        xbig1 = keep.tile([128, 2, FD], f16, name="xbig1")
        nc.gpsimd.dma_start(
            out=xbig1[:],
            in_=feat[256:512, :].rearrange("(a p) f -> p a f", p=128))

        def xchunk(kc):
            return xbig0[:, kc, :] if kc < 2 else xbig1[:, kc - 2, :]

        iota1024 = consts.tile([128, NSEG], f32)
        nc.gpsimd.iota(iota1024[:], [[1, NSEG]], channel_multiplier=0,
                       allow_small_or_imprecise_dtypes=True)
        # biota[j] = 32 - j  (for first-argmax via reduce_max)
        biota = consts.tile([128, C], f32)
        nc.gpsimd.iota(biota[:], [[-1, C]], base=C, channel_multiplier=0,
                       allow_small_or_imprecise_dtypes=True)

        NST = 2 * D * C + 2   # 130 stats columns: [lq | p | ones | E]
        es_oh = [keep.tile([128, NSEG], f16, name=f"esoh_{kc}")
                 for kc in range(KT)]
        es_st = [keep.tile([128, NST], f16, name=f"esst_{kc}")
                 for kc in range(KT)]
        ef_16 = [keep.tile([128, EF], f16, name=f"eff_{kc}")
                 for kc in range(KT)]

        # ---- row sum-of-squares: squares on gpsimd (otherwise idle; keeps
        # ACT free for the Ln-first ordering), one batched DVE reduce
        # (gpsimd cannot reduce along the free axis) ----
        sqpack = keep.tile([128, KT], f32, name="sqpack")
        scrg4 = keep.tile([128, KT, FD], f32, name="scrg4")
        for kc in range(KT):
            nc.gpsimd.tensor_tensor(out=scrg4[:, kc, :], in0=xchunk(kc),
                                    in1=xchunk(kc), op=Alu.mult)
        nc.vector.reduce_sum(out=sqpack[:], in_=scrg4[:], axis=Ax.X)

        # ---- targets chains (DVE) ----
        # es_st columns: [0:64 lq | 64:128 p | 128 ones | 129 E]
        t1big = keep.tile([128, KT, D * C], f32, name="t1big")
        nc.vector.tensor_scalar_add(out=t1big[:], in0=tbig[:], scalar1=1e-10)
        invsb = keep.tile([128, KT * D], f32, name="invsb")
        nc.vector.reduce_sum(
            out=invsb[:],
            in_=t1big[:].rearrange("p a (d c) -> p (a d) c", c=C),
            axis=Ax.X)
        nc.vector.reciprocal(invsb[:], invsb[:])

        # ACT phase 1: Ln table preload (dummy) then the 4 Lns — the stats
        # m-tiles are the critical path (they feed the early AllReduce)
        lnscr = work.tile([128, 1], f32, name="lnscr", tag="lnscr")
        act_chain = [nc.scalar.activation(out=lnscr[:], in_=ones128[:],
                                          func=Act.Ln)]
        ln_acts = []
        for kc in range(KT):
            st_t = es_st[kc]
            pt = st_t[:, D * C:2 * D * C]
            for d_ in range(D):
                nc.vector.tensor_scalar_mul(
                    out=pt[:, C * d_:C * (d_ + 1)],
                    in0=t1big[:, kc, C * d_:C * (d_ + 1)],
                    scalar1=invsb[:, kc * D + d_:kc * D + d_ + 1])
            ln_acts.append(nc.scalar.activation(
                out=st_t[:, 0:D * C], in_=pt, func=Act.Ln))

        # E / ones columns right after each chunk's Ln
        for kc in range(KT):
            st_t = es_st[kc]
            scr64 = work.tile([128, D * C], f32, name=f"scr64_{kc}",
                              tag=f"s64_{kc}")
            nc.vector.tensor_tensor(out=scr64[:],
                                    in0=st_t[:, D * C:2 * D * C],
                                    in1=st_t[:, 0:D * C], op=Alu.mult)
            escr = work.tile([128, 1], f32, name=f"escr_{kc}",
                             tag=f"es_{kc}")
            nc.vector.reduce_sum(out=escr[:], in_=scr64[:], axis=Ax.X)
            nc.vector.tensor_copy(out=st_t[:, NST - 1:NST], in_=escr[:])
            nc.vector.memset(st_t[:, NST - 2:NST - 1], 1.0)

        # ---- 1/norm: one ACT Sqrt + DVE reciprocal (phase 2) ----
        nc.vector.tensor_scalar_max(out=sqpack[:], in0=sqpack[:],
                                    scalar1=1e-24)
        normpack = keep.tile([128, KT], f32, name="normpack")
        act_chain.append(nc.scalar.sqrt(normpack[:], sqpack[:]))
        invpack = keep.tile([128, KT], f32, name="invpack")
        nc.vector.reciprocal(invpack[:], normpack[:])
        minvpack = keep.tile([128, KT], f32, name="minvpack")
        nc.vector.tensor_tensor(out=minvpack[:], in0=invpack[:],
                                in1=mkbig[:, :, 0], op=Alu.mult)

        # ---- ext_feat = [x*(m*inv) | m | sq*inv*minv] (ACT phase 3) ----
        copy_acts = []
        for kc in range(KT):
            ef_t = ef_16[kc]
            copy_acts.append(nc.scalar.activation(
                out=ef_t[:, 0:FD], in_=xchunk(kc), func=Act.Copy,
                scale=minvpack[:, kc:kc + 1]))
            nc.vector.tensor_copy(out=ef_t[:, FD:FD + 1], in_=mkbig[:, kc, :])
            nc.vector.tensor_scalar(out=ef_t[:, FD + 1:FD + 2],
                                    in0=sqpack[:, kc:kc + 1],
                                    scalar1=invpack[:, kc:kc + 1],
                                    scalar2=minvpack[:, kc:kc + 1],
                                    op0=Alu.mult, op1=Alu.mult)

        # ---- chunk-batched first-argmax, then code = cls0 + 32*cls1 ----
        AD = KT * D   # 8 (kc, d) groups
        mx8 = work.tile([128, AD], f32, name="mx8", tag="mx8")
        nc.vector.reduce_max(
            out=mx8[:],
            in_=t1big[:].rearrange("p a (d c) -> p (a d) c", c=C),
            axis=Ax.X)
        cand8 = work.tile([128, AD, C], f32, name="cand8", tag="cand8")
        for kc in range(KT):
            for d_ in range(D):
                g = kc * D + d_
                # (t1 == max) * (32 - idx); reduce_max -> 32 - first_argmax
                nc.vector.scalar_tensor_tensor(
                    out=cand8[:, g, :],
                    in0=t1big[:, kc, C * d_:C * (d_ + 1)],
                    scalar=mx8[:, g:g + 1], in1=biota[:],
                    op0=Alu.is_equal, op1=Alu.mult)
        mq8 = work.tile([128, AD], f32, name="mq8", tag="mq8")
        nc.vector.reduce_max(out=mq8[:], in_=cand8[:], axis=Ax.X)
        cls8 = work.tile([128, AD], f32, name="cls8", tag="cls8")
        nc.vector.tensor_scalar(out=cls8[:], in0=mq8[:], scalar1=-1.0,
                                scalar2=float(C), op0=Alu.mult, op1=Alu.add)
        # code4[kc] = cls[kc,0] + 32*cls[kc,1]
        code4 = work.tile([128, KT], f32, name="code4", tag="code4")
        cls_v = cls8[:].rearrange("p (a two) -> p a two", two=2)
        nc.vector.tensor_scalar_mul(out=code4[:], in0=cls_v[:, :, 1],
                                    scalar1=float(C))
        nc.vector.tensor_tensor(out=code4[:], in0=code4[:],
                                in1=cls_v[:, :, 0], op=Alu.add)
        for kc in range(KT):
            nc.vector.tensor_scalar(
                out=es_oh[kc][:], in0=iota1024[:],
                scalar1=code4[:, kc:kc + 1],
                scalar2=None, op0=Alu.is_equal)

        # keep ACT ops grouped by function (avoid act-table reload thrash)
        act_chain = (act_chain[:1] + ln_acts + act_chain[1:] + copy_acts)
        for a, b in zip(act_chain[1:], act_chain[:-1]):
            add_dep_helper(a.ins, b.ins, sync=False,
                           reason="act table grouping")

        # ---------------- payload tiles + AllReduce buffers ----------------
        # stats packed [64, 3, 258]: slot0 = LQ rows, slot1 = P rows (pair i
        # on partition i), slot2 = F row (p0) + E row (p1), rest zero.
        seg_pay = keep.tile([128, 8, EF], f16, name="seg_pay")
        st_pay = keep.tile([64, 3, EF], f32, name="st_pay")
        nc.vector.memset(st_pay[:, 2:3, :], 0.0)
        inb_st = dram.tile([65, 2, EF], f32, name="inb_st")
        outb_st = dram.tile([65, 2, EF], f32, name="outb_st",
                            addr_space="Shared")
        inb_seg = dram.tile([128, 8, EF], f16, name="inb_seg")
        outb_seg = dram.tile([128, 8, EF], f16, name="outb_seg",
                             addr_space="Shared")

        # ---------------- matmuls ----------------------
        # Both payloads are ready long before the wall-clock floor (~55us)
        # at which the first mesh can begin, so order the ARs for epilogue
        # overlap: segments FIRST (their heavy epilogue hides inside the
        # stats mesh), packed stats second.
        for mt in range(8):
            mlo = mt * 128
            ps = psum.tile([128, EF], f32, name=f"ps_{mt}", tag=f"ps_{mt % 7}")
            for kc in range(KT):
                nc.tensor.matmul(out=ps[:], lhsT=es_oh[kc][:, mlo:mlo + 128],
                                 rhs=ef_16[kc][:],
                                 start=(kc == 0), stop=(kc == KT - 1))
            # alternate engines so copies keep pace with the matmuls
            # (gpsimd cannot read PSUM; scalar's Copy is table-less)
            if mt % 2 == 0:
                nc.vector.tensor_copy(out=seg_pay[:, mt, :], in_=ps[:])
            else:
                nc.scalar.activation(out=seg_pay[:, mt, :], in_=ps[:],
                                     func=Act.Copy)
        nc.sync.dma_start(out=inb_seg[:], in_=seg_pay[:])
        nc.gpsimd.collective_compute(
            "AllReduce", mybir.AluOpType.add,
            replica_groups=[list(range(NCORES))],
            ins=[inb_seg.opt()], outs=[outb_seg.opt()])

        # stats m-tiles: LQ and P as separate m=64 chunks so the pair rows
        # land partition-aligned in slots 0/1 (no re-basing DMA later)
        psA = psum.tile([64, EF], f32, name="psA", tag="ps_0")
        psB = psum.tile([64, EF], f32, name="psB", tag="ps_1")
        psC = psum.tile([2, EF], f32, name="psC", tag="ps_2")
        for kc in range(KT):
            st = (kc == 0)
            sp = (kc == KT - 1)
            nc.tensor.matmul(out=psA[:], lhsT=es_st[kc][:, 0:64],
                             rhs=ef_16[kc][:], start=st, stop=sp)
            nc.tensor.matmul(out=psB[:], lhsT=es_st[kc][:, 64:128],
                             rhs=ef_16[kc][:], start=st, stop=sp)
            nc.tensor.matmul(out=psC[:], lhsT=es_st[kc][:, 128:130],
                             rhs=ef_16[kc][:], start=st, stop=sp)
        nc.vector.tensor_copy(out=st_pay[:, 0, :], in_=psA[:])
        nc.vector.tensor_copy(out=st_pay[:, 1, :], in_=psB[:])
        nc.vector.tensor_copy(out=st_pay[0:2, 2, :], in_=psC[0:2, :])
        nc.sync.dma_start(out=inb_st[0:64, :, :], in_=st_pay[:, 0:2, :])
        # F/E rows (partitions 0/1 of slot 2) flatten into DRAM row 64
        nc.sync.dma_start(out=inb_st[64:65, 0:2, :], in_=st_pay[0:2, 2, :])
        nc.gpsimd.collective_compute(
            "AllReduce", mybir.AluOpType.add,
            replica_groups=[list(range(NCORES))],
            ins=[inb_st.opt()], outs=[outb_st.opt()])

        # ---------------- segment epilogue (hides in the stats mesh) ------
        # loads split over two queues; squares split ACT/DVE
        big0 = keep.tile([128, 4, EF], f16, name="big0")
        nc.sync.dma_start(out=big0[:], in_=outb_seg[:, 0:4, :])
        big1 = keep.tile([128, 4, EF], f16, name="big1")
        nc.scalar.dma_start(out=big1[:], in_=outb_seg[:, 4:8, :])

        Z = keep.tile([128, 8], f32, name="Z")
        nc.vector.memset(Z[:], 0.0)

        nrmp = keep.tile([128, 8], f32, name="nrmp")
        sq_acts = []
        for s in range(4):
            sq_acts.append(nc.scalar.activation(
                out=scrg4[:, 0, :], in_=big1[:, s, 0:FD], func=Act.Square,
                accum_out=nrmp[:, 4 + s:5 + s]))
        for a, b in zip(sq_acts[1:], sq_acts[:-1]):
            add_dep_helper(a.ins, b.ins, sync=False, reason="act grouping")
        scrB = keep.tile([128, 4, FD], f32, name="scrB")
        nc.vector.tensor_tensor(out=scrB[:], in0=big0[:, :, 0:FD],
                                in1=big0[:, :, 0:FD], op=Alu.mult)
        nc.vector.reduce_sum(out=nrmp[:, 0:4], in_=scrB[:], axis=Ax.X)
        cdp = keep.tile([128, 8], f32, name="cdp")
        nc.vector.tensor_scalar_max(out=cdp[:, 0:4], in0=big0[:, :, FD],
                                    scalar1=1.0)
        nc.vector.tensor_scalar_max(out=cdp[:, 4:8], in0=big1[:, :, FD],
                                    scalar1=1.0)
        rcdp = keep.tile([128, 8], f32, name="rcdp")
        nc.vector.reciprocal(rcdp[:], cdp[:])
        termp = keep.tile([128, 8], f32, name="termp")
        nc.vector.tensor_tensor(out=termp[:], in0=nrmp[:], in1=rcdp[:],
                                op=Alu.mult)
        nc.vector.reduce_sum(out=Z[:, 0:1], in_=termp[:], axis=Ax.X)

        # ---------------- stats epilogue (after the second mesh) ----------
        stall = keep.tile([64, 2, EF], f32, name="stall")
        nc.sync.dma_start(out=stall[:], in_=outb_st[0:64, :, :])
        frow2 = keep.tile([1, 2, EF], f32, name="frow2")
        nc.sync.dma_start(out=frow2[:], in_=outb_st[64:65, :, :])
        # F and E feature rows transposed to 128 partitions x 2 so Fe.F is a
        # lane-parallel multiply that rides the ones-matmul (columns 4:6)
        frT = keep.tile([128, 2], f32, name="frT")
        nc.scalar.dma_start(
            out=frT[:],
            in_=outb_st[64:65, 0, 0:FD].rearrange("o (a p) -> p (o a)", p=128))
        erT = keep.tile([128, 2], f32, name="erT")
        nc.scalar.dma_start(
            out=erT[:],
            in_=outb_st[64:65, 1, 0:FD].rearrange("o (a p) -> p (o a)", p=128))
        ut = stall[:, 0, :]
        vt = stall[:, 1, :]
        frow = frow2[0:1, 0, :]
        erow = frow2[0:1, 1, :]

        scrU = keep.tile([64, FD], f32, name="scrU")
        nc.vector.tensor_tensor(out=scrU[:], in0=ut[:, 0:FD],
                                in1=vt[:, 0:FD], op=Alu.mult)
        nc.vector.reduce_sum(out=Z[0:64, 1:2], in_=scrU[:], axis=Ax.X)
        nc.vector.tensor_tensor(out=Z[0:64, 2:3], in0=vt[:, FD + 1:FD + 2],
                                in1=ut[:, FD:FD + 1], op=Alu.mult)     # Psq*L
        nc.vector.tensor_tensor(out=Z[0:64, 3:4], in0=vt[:, FD:FD + 1],
                                in1=ut[:, FD + 1:FD + 2], op=Alu.mult)  # Pbar*Lsq
        nc.vector.tensor_tensor(out=Z[:, 4:6], in0=frT[:],
                                in1=erT[:], op=Alu.mult)               # Fe.F

        zred = psum.tile([1, 8], f32, name="zred", tag="ps_3")
        nc.tensor.matmul(out=zred[:], lhsT=ones128[:], rhs=Z[:],
                         start=True, stop=True)
        zs = keep.tile([1, 8], f32, name="zs")
        nc.vector.tensor_copy(out=zs[:], in_=zred[:])

        # scalars: M=F[256], a=F[257], e=E[256], se=E[257] (all fp32)
        Mv = frow[0:1, FD:FD + 1]
        av = frow[0:1, FD + 1:FD + 2]
        ev = erow[0:1, FD:FD + 1]
        sev = erow[0:1, FD + 1:FD + 2]
        s_center = zs[0:1, 0:1]
        uv = zs[0:1, 1:2]
        psql = zs[0:1, 2:3]
        pbarlsq = zs[0:1, 3:4]
        fef = zs[0:1, 6:7]
        nc.vector.tensor_tensor(out=fef, in0=zs[0:1, 4:5],
                                in1=zs[0:1, 5:6], op=Alu.add)

        fin = keep.tile([1, 16], f32, name="fin")
        t_ = lambda i: fin[0:1, i:i + 1]
        # f0 = se*M ; f1 = a*e ; f2 = f0+f1
        nc.vector.tensor_tensor(out=t_(8), in0=sev, in1=Mv, op=Alu.mult)
        nc.vector.tensor_tensor(out=t_(9), in0=av, in1=ev, op=Alu.mult)
        nc.vector.tensor_tensor(out=t_(10), in0=t_(8), in1=t_(9), op=Alu.add)
        # f3 = -2*fef + f2
        nc.vector.tensor_scalar(out=t_(11), in0=fef, scalar1=-2.0,
                                scalar2=t_(10), op0=Alu.mult, op1=Alu.add)
        # f4 = f3 - psql ; f5 = f4 - pbarlsq
        nc.vector.tensor_tensor(out=t_(12), in0=t_(11), in1=psql, op=Alu.subtract)
        nc.vector.tensor_tensor(out=t_(13), in0=t_(12), in1=pbarlsq, op=Alu.subtract)
        # SD = 2*uv + f5
        nc.vector.tensor_scalar(out=t_(14), in0=uv, scalar1=2.0,
                                scalar2=t_(13), op0=Alu.mult, op1=Alu.add)
        # md = M*(M-1) ; rmd = 1/md ; div = SD*rmd*(-1/D)
        nc.vector.tensor_scalar(out=t_(15), in0=Mv, scalar1=-1.0,
                                scalar2=Mv, op0=Alu.add, op1=Alu.mult)
        nc.vector.reciprocal(t_(15), t_(15))
        nc.vector.tensor_tensor(out=t_(1), in0=t_(14), in1=t_(15), op=Alu.mult)
        nc.vector.tensor_scalar_mul(out=t_(1), in0=t_(1), scalar1=-1.0 / D)
        # tight = (a - s_center)/M
        nc.vector.tensor_tensor(out=t_(7), in0=av, in1=s_center, op=Alu.subtract)
        nc.vector.reciprocal(t_(6), Mv)
        nc.vector.tensor_tensor(out=t_(2), in0=t_(7), in1=t_(6), op=Alu.mult)
        # total = 0.1*div + 0.1*tight
        nc.vector.tensor_tensor(out=t_(0), in0=t_(1), in1=t_(2), op=Alu.add)
        nc.vector.tensor_scalar_mul(out=t_(0), in0=t_(0), scalar1=0.1)
        nc.sync.dma_start(out=outd[None, :], in_=fin[0:1, 0:8])

    nc.finalize()
    return nc


def _get_compiled():
    if "nc" not in _compiled:
        _compiled["nc"] = _build_bass()
    return _compiled["nc"]


def _make_in_maps(features, targets, mask):
    features = np.ascontiguousarray(np.asarray(features).astype(np.float16))
    targets = np.ascontiguousarray(np.asarray(targets, dtype=np.float32))
    maskf = np.asarray(mask).astype(np.float32).reshape(B, 1)
    in_maps = []
    for i in range(NCORES):
        sl = slice(i * RB, (i + 1) * RB)
        in_maps.append({
            "features": features[sl],
            "targets": targets[sl],
            "maskf": np.ascontiguousarray(maskf[sl]),
        })
    return in_maps


def kernel(features, targets, mask):
    from concourse.bass_utils import run_bass_kernel_spmd

    nc = _get_compiled()
    in_maps = _make_in_maps(features, targets, mask)
    res = run_bass_kernel_spmd(nc, in_maps, list(range(NCORES)))
    out = res.results[0]["out"]
    total = np.float32(out[0])
    diversity = np.float32(out[1])
    tightness = np.float32(out[2])
    return total, diversity, tightness
